# revision 2
# baseline (speedup 1.0000x reference)
"""Trainium2 Bass kernel for nn_Gtu (gated Toeplitz unit / TNN GTU layer).

  uv = silu(x @ W_uv); u, v = split(uv)
  t  = RPE-MLP(arange(n)) * gamma^k          (per-channel causal Toeplitz coefs)
  o  = causal_conv(t, v)                     (per channel, via length-8192 FFT)
  out = (u * o) @ W_o

8 cores = (batch 0..3) x (d1-half 0..1). Each core handles its batch and 512
channels end-to-end plus the partial output projection; the host sums the two
partials per batch (row-split of W_o) and concatenates batches.

FFT: four-step matmul factorization, L = 8192 = 128 x 64:
  n = n1*64 + n2 (n1 in [0,128) contracted; input support n1 < 64)
  k = k1 + 128*k2
  A[k1,(q,s,n2)] = sum_n1 in[n1] W128^(n1 k1)            (stage A, PE)
  B = A * W8192^(k1 n2)                                  (twiddle, DVE/GP)
  per-tau transpose [k1,(s,n2)] -> [(s,n2),k1]           (PE)
  Z[(s,k2),(q,k1)] = sum_n2 B^T W64^(n2 k2) (block-diag) (stage B, PE)
Real channels packed in pairs z = v_c + i*v_{c+1} (s in {0,1} per tile; a
"quad" = 4 tiles = 16 real channels, free dim 512). Spectral multiply:
  e1 = Wre+Wim, f = Wre-Wim (W = packed t-spectrum)
  S4 = (e1 + rev f) + i(rev f - f);  D4 = (f + rev e1) + i(e1 - rev e1)
  Q4 = Z*S4 + conj(rev Z)*D4        (= 4*(Zc Tc + i Zc1 Tc1) packed product)
Inverse mirrors forward; 1/(4L) is folded into the final inverse DFT matrix.
Re/Im of the inverse are o_c / o_{c+1}.
"""

import numpy as np
import ml_dtypes

import concourse.bass as bass
import concourse.tile as tile
import concourse.mybir as mybir
from concourse import bacc
from concourse.bass_utils import run_bass_kernel_spmd

F32 = mybir.dt.float32
F32R = mybir.dt.float32r
BF16 = mybir.dt.bfloat16
AF = mybir.ActivationFunctionType
ALU = mybir.AluOpType
AXX = mybir.AxisListType.X

B, N, D = 4, 4096, 1024
D1 = 1024
H = 512
L = 8192
FEAT = 32
RPE_LAYERS = 3
LOWER = 0.99
LN_EPS = 1e-5
NQUAD = 32

_NP_BF16 = ml_dtypes.bfloat16


def _host_consts():
    c = {}
    bf = lambda a: np.ascontiguousarray(a, dtype=_NP_BF16)
    f32 = lambda a: np.ascontiguousarray(a, dtype=np.float32)

    n1 = np.arange(64)[:, None]
    k1 = np.arange(128)[None, :]
    w = np.exp(-2j * np.pi * n1 * k1 / 128.0)
    c["fa_re"], c["fa_im"], c["fa_imn"] = bf(w.real), bf(w.imag), bf(-w.imag)

    k1c = np.arange(128)[:, None]
    n2c = np.arange(64)[None, :]
    tw = np.tile(np.exp(-2j * np.pi * k1c * n2c / float(L)), (1, 8))
    c["tw_re"], c["tw_im"] = bf(tw.real), bf(tw.imag)

    n2 = np.arange(64)[:, None]
    k2 = np.arange(64)[None, :]
    g = np.exp(-2j * np.pi * n2 * k2 / 64.0)
    gb = np.zeros((128, 128), np.complex128)
    gb[:64, :64] = g
    gb[64:, 64:] = g
    c["g_re"], c["g_im"], c["g_imn"] = bf(gb.real), bf(gb.imag), bf(-gb.imag)
    gi = np.conj(gb)
    c["gi_re"], c["gi_im"], c["gi_imn"] = bf(gi.real), bf(gi.imag), bf(-gi.imag)

    n2r = np.arange(64)[:, None]
    k1r = np.arange(128)[None, :]
    ti = np.exp(+2j * np.pi * n2r * k1r / float(L))
    tit = np.tile(np.concatenate([ti, ti], axis=0), (1, 4))
    c["ti_re"], c["ti_im"] = bf(tit.real), bf(tit.imag)

    k1f = np.arange(128)[:, None]
    n1f = np.arange(64)[None, :]
    fi = np.exp(+2j * np.pi * k1f * n1f / 128.0) / (4.0 * L)
    c["fi_re"], c["fi_im"], c["fi_imn"] = bf(fi.real), bf(fi.imag), bf(-fi.imag)

    # reversal stage-B variants: Zrev[k2'] uses G columns 63-k2' (main) and
    # (64-k2')%64 (the k1=0 column); t-side combos produce e1/f/e1r/fr directly
    def blockdiag(m):
        out = np.zeros((128, 128), np.complex128)
        out[:64, :64] = m
        out[64:, 64:] = m
        return out
    n2v = np.arange(64)[:, None]
    k2v = np.arange(64)[None, :]
    grev = blockdiag(np.exp(-2j * np.pi * n2v * (63 - k2v) / 64.0))
    grev0 = blockdiag(np.exp(-2j * np.pi * n2v * ((64 - k2v) % 64) / 64.0))
    c["grev_re"], c["grev_im"], c["grev_imn"] = bf(grev.real), bf(grev.imag), bf(-grev.imag)
    c["grev0_re"], c["grev0_im"], c["grev0_imn"] = bf(grev0.real), bf(grev0.imag), bf(-grev0.imag)
    c["ge0"] = bf(gb.real + gb.imag)
    c["gf0"] = bf(gb.real - gb.imag)
    c["nge0"] = bf(-(gb.real + gb.imag))
    c["ge1"] = bf(grev.real + grev.imag)
    c["gf1"] = bf(grev.real - grev.imag)
    c["nge1"] = bf(-(grev.real + grev.imag))
    c["ge1c"] = bf(grev0.real + grev0.imag)
    c["gf1c"] = bf(grev0.real - grev0.imag)
    c["nge1c"] = bf(-(grev0.real + grev0.imag))

    c["ident"] = bf(np.eye(128))
    c["ident_f32"] = f32(np.eye(128))

    p = np.arange(128)
    c["idxmat"] = f32(p[:, None] + 128.0 * np.arange(32)[None, :])
    c["pbc"] = f32(np.tile(p[None, :], (128, 1)))
    return c


CONSTS = _host_consts()


def build_program(nc):
    x = nc.dram_tensor("x", [N, D], F32, kind="ExternalInput")
    wuv = nc.dram_tensor("wuv", [D, 2 * H], F32, kind="ExternalInput")
    wo = nc.dram_tensor("wo", [H, D], F32, kind="ExternalInput")
    rpeo = nc.dram_tensor("rpeo", [FEAT, H], F32, kind="ExternalInput")
    dg = nc.dram_tensor("dg", [1, H], F32, kind="ExternalInput")
    rpe_in = nc.dram_tensor("rpe_in", [1, FEAT], F32, kind="ExternalInput")
    rpe_hid = nc.dram_tensor("rpe_hid", [RPE_LAYERS, FEAT, FEAT], F32,
                             kind="ExternalInput")
    ln_g = nc.dram_tensor("ln_g", [RPE_LAYERS, FEAT], F32, kind="ExternalInput")
    ln_b = nc.dram_tensor("ln_b", [RPE_LAYERS, FEAT], F32, kind="ExternalInput")

    cds = {}
    for name, arr in CONSTS.items():
        dt = BF16 if arr.dtype == _NP_BF16 else F32
        cds[name] = nc.dram_tensor(name, list(arr.shape), dt, kind="ExternalInput")

    t_bf = nc.dram_tensor("t_bf", [H, N], BF16, kind="Internal")
    v_bf = nc.dram_tensor("v_bf", [H, N], BF16, kind="Internal")
    u_bf = nc.dram_tensor("u_bf", [H, N], BF16, kind="Internal")
    o_bf = nc.dram_tensor("o_bf", [H, N], BF16, kind="Internal")
    lng_dram = nc.dram_tensor("lng_dram", [1, H], F32, kind="Internal")
    out = nc.dram_tensor("out", [D, N], F32, kind="ExternalOutput")

    ap = lambda t, off, pattern: bass.AP(tensor=t, offset=off, ap=pattern)

    with tile.TileContext(nc) as tc:
        with tc.tile_pool(name="consts", bufs=1) as cp:
            cs = {}
            for name, arr in CONSTS.items():
                dt = BF16 if arr.dtype == _NP_BF16 else F32
                ct = cp.tile(list(arr.shape), dt, tag=f"c_{name}")
                nc.sync.dma_start(out=ct, in_=cds[name][:, :])
                cs[name] = ct
            ident_r = cp.tile([128, 128], F32R, tag="ident_r")
            nc.gpsimd.dma_start(out=ident_r, in_=cds["ident_f32"][:, :])

            w_in_bc = cp.tile([128, FEAT], F32, tag="w_in_bc")
            nc.sync.dma_start(out=w_in_bc, in_=ap(rpe_in, 0, [[0, 128], [1, FEAT]]))
            lng_bc, lnb_bc = [], []
            for l in range(RPE_LAYERS):
                g_t = cp.tile([128, FEAT], F32, tag=f"lng{l}")
                b_t = cp.tile([128, FEAT], F32, tag=f"lnb{l}")
                nc.sync.dma_start(out=g_t, in_=ap(ln_g, l * FEAT, [[0, 128], [1, FEAT]]))
                nc.sync.dma_start(out=b_t, in_=ap(ln_b, l * FEAT, [[0, 128], [1, FEAT]]))
                lng_bc.append(g_t)
                lnb_bc.append(b_t)

            whid = []
            for l in range(RPE_LAYERS):
                wt4 = cp.tile([128, 128], F32, tag=f"whid{l}")
                nc.vector.memset(wt4, 0.0)
                for j in range(4):
                    nc.sync.dma_start(
                        out=wt4[32 * j:32 * j + 32, 32 * j:32 * j + 32],
                        in_=rpe_hid[l, :, :])
                whid.append(wt4)

            eps_t = cp.tile([128, 1], F32, tag="eps_t")
            nc.vector.memset(eps_t, LN_EPS)
            rpeo_sb = cp.tile([128, H], F32, tag="rpeo_sb")
            for j in range(4):
                nc.sync.dma_start(out=rpeo_sb[32 * j:32 * j + 32, :], in_=rpeo[:, :])

            # decay -> lngam_col [128, 4]  (lngam_col[cp, cb] = ln gamma_{128cb+cp})
            with tc.tile_pool(name="dk", bufs=1) as dk:
                dg_sb = dk.tile([1, H], F32, tag="dg")
                nc.sync.dma_start(out=dg_sb, in_=dg[:, :])
                sig = dk.tile([1, H], F32, tag="sig")
                nc.scalar.activation(sig, dg_sb, AF.Sigmoid)
                gam = dk.tile([1, H], F32, tag="gam")
                nc.vector.tensor_scalar(gam, sig, 1.0 - LOWER, LOWER,
                                        ALU.mult, ALU.add)
                lngr = dk.tile([1, H], F32, tag="lngr")
                nc.scalar.activation(lngr, gam, AF.Ln)
                nc.sync.dma_start(out=lng_dram[:, :], in_=lngr)
            lngam_col = cp.tile([128, 4], F32, tag="lngam_col")
            nc.sync.dma_start(out=lngam_col,
                              in_=ap(lng_dram, 0, [[1, 128], [128, 4]]))

            # =====================================================
            # Phase A: RPE MLP -> t_bf (channel-major bf16)
            # =====================================================
            idxm = cs["idxmat"]
            with tc.tile_pool(name="rpe", bufs=2) as rp, \
                 tc.tile_pool(name="rpe_ps", bufs=2, space="PSUM") as rps:
                for grp in range(8):
                    h_sm = rp.tile([128, 4, FEAT], F32, tag="h_sm")
                    for jj in range(4):
                        j = 4 * grp + jj
                        nc.scalar.activation(h_sm[:, jj, :], w_in_bc, AF.Silu,
                                             scale=idxm[:, j:j + 1])
                    h_fm = None
                    for l in range(RPE_LAYERS):
                        mu = rp.tile([128, 4], F32, tag="mu")
                        nc.vector.tensor_reduce(mu, h_sm, AXX, ALU.add)
                        nc.vector.tensor_scalar_mul(mu, mu, 1.0 / FEAT)
                        hc = rp.tile([128, 4, FEAT], F32, tag="hc")
                        nc.vector.tensor_tensor(
                            hc, h_sm, mu[:, :, None].to_broadcast((128, 4, FEAT)),
                            ALU.subtract)
                        sq = rp.tile([128, 4, FEAT], F32, tag="sq")
                        nc.scalar.activation(sq, hc, AF.Square)
                        var = rp.tile([128, 4], F32, tag="var")
                        nc.vector.tensor_reduce(var, sq, AXX, ALU.add)
                        rstd = rp.tile([128, 4], F32, tag="rstd")
                        nc.scalar.activation(rstd, var, AF.Sqrt,
                                             scale=1.0 / FEAT, bias=eps_t)
                        nc.vector.reciprocal(rstd, rstd)
                        hn = rp.tile([128, 4, FEAT], F32, tag="hn")
                        nc.vector.tensor_tensor(
                            hn, hc, rstd[:, :, None].to_broadcast((128, 4, FEAT)),
                            ALU.mult)
                        gb_ = lng_bc[l][:, None, :].to_broadcast((128, 4, FEAT))
                        bb_ = lnb_bc[l][:, None, :].to_broadcast((128, 4, FEAT))
                        hs = rp.tile([128, 4, FEAT], F32, tag="hs")
                        nc.vector.tensor_tensor(hs, hn, gb_, ALU.mult)
                        nc.vector.tensor_tensor(hs, hs, bb_, ALU.add)
                        pt = rps.tile([128, 128], F32, tag="tp")
                        nc.tensor.transpose(
                            pt, hs.rearrange("p a b -> p (a b)"), cs["ident_f32"])
                        ln_fm = rp.tile([128, 128], F32, tag="ln_fm")
                        nc.scalar.activation(ln_fm, pt, AF.Copy)
                        hp = rps.tile([128, 128], F32, tag="mm")
                        nc.tensor.matmul(hp, whid[l], ln_fm, start=True, stop=True)
                        h_fm = rp.tile([128, 128], F32, tag="h_fm")
                        nc.scalar.activation(h_fm, hp, AF.Silu)
                        if l < RPE_LAYERS - 1:
                            pt2 = rps.tile([128, 128], F32, tag="tp")
                            nc.tensor.transpose(pt2, h_fm, cs["ident_f32"])
                            nc.scalar.activation(
                                h_sm.rearrange("p a b -> p (a b)"), pt2, AF.Copy)
                    for jj in range(4):
                        j = 4 * grp + jj
                        for cb in range(4):
                            tp = rps.tile([128, 128], F32, tag="tmm")
                            nc.tensor.matmul(
                                tp,
                                rpeo_sb[32 * jj:32 * jj + 32,
                                        128 * cb:128 * cb + 128],
                                h_fm[32 * jj:32 * jj + 32, :],
                                start=True, stop=True,
                                tile_position=(32 * jj, 0))
                            ebias = rp.tile([128, 1], F32, tag="ebias")
                            nc.vector.tensor_scalar_mul(
                                ebias, lngam_col[:, cb:cb + 1], float(128 * j))
                            ee = rp.tile([128, 128], F32, tag="ee")
                            nc.scalar.activation(ee, cs["pbc"], AF.Exp,
                                                 scale=lngam_col[:, cb:cb + 1],
                                                 bias=ebias)
                            tt = rp.tile([128, 128], BF16, tag="t_out")
                            nc.vector.tensor_tensor(tt, tp, ee, ALU.mult)
                            nc.sync.dma_start(
                                out=ap(t_bf, 128 * cb * N + 128 * j,
                                       [[N, 128], [1, 128]]),
                                in_=tt)

            # =====================================================
            # Phase B: uv projection (f32r) + silu -> u_bf, v_bf
            # =====================================================
            with tc.tile_pool(name="pb_w", bufs=1) as wbp, \
                 tc.tile_pool(name="pb", bufs=2) as pb, \
                 tc.tile_pool(name="pb_ps", bufs=2, space="PSUM") as pps:
                wuv_sb = wbp.tile([128, 8, 2 * H], F32R, tag="wuv_sb")
                nc.gpsimd.dma_start(
                    out=wuv_sb,
                    in_=ap(wuv, 0, [[2 * H, 128], [128 * 2 * H, 8], [1, 2 * H]]))
                for stg in range(8):
                    xT = pb.tile([128, 8, 512], F32R, tag="xT")
                    for st4 in range(4):
                        st = 4 * stg + st4
                        xt = pb.tile([128, D], F32R, tag="x_in")
                        nc.gpsimd.dma_start(out=xt,
                                            in_=x[128 * st:128 * st + 128, :])
                        for k in range(8):
                            ptx = pps.tile([128, 128], F32R, tag="x_tp")
                            nc.tensor.transpose(
                                ptx, xt[:, 128 * k:128 * k + 128], ident_r)
                            nc.scalar.activation(
                                xT[:, k, 128 * st4:128 * st4 + 128], ptx, AF.Copy)
                    for cblk in range(8):
                        pu = pps.tile([128, 512], F32, tag="uv_mm")
                        for k in range(8):
                            nc.tensor.matmul(
                                pu, wuv_sb[:, k, 128 * cblk:128 * cblk + 128],
                                xT[:, k, :], start=(k == 0), stop=(k == 7))
                        uv_sb = pb.tile([128, 512], BF16, tag="uv_out")
                        nc.scalar.activation(uv_sb, pu, AF.Silu)
                        dst = u_bf if cblk < 4 else v_bf
                        cbase = (cblk % 4) * 128
                        nc.sync.dma_start(
                            out=ap(dst, cbase * N + 512 * stg,
                                   [[N, 128], [1, 512]]),
                            in_=uv_sb)

            # =====================================================
            # Phase C: FFT conv, one quad (16 channels) at a time
            # =====================================================
            with tc.tile_pool(name="pc", bufs=2) as pc, \
                 tc.tile_pool(name="pc_sp", bufs=2) as sp, \
                 tc.tile_pool(name="pc_ps", bufs=1, space="PSUM") as ps, \
                 tc.tile_pool(name="pc_pst", bufs=2, space="PSUM") as pst:

                def stage_a_and_transpose(src_dram, q4, pfx):
                    """DMA-gather + stage A + twiddle + per-tau transpose.
                    Returns (bt_re, bt_im) sbuf bf16 [128,(q,k1)=512]."""
                    rr = pc.tile([64, 512], BF16, tag=f"{pfx}_rhs_re")
                    ri = pc.tile([64, 512], BF16, tag=f"{pfx}_rhs_im")
                    base = 16 * q4 * N
                    pat = [[64, 64], [4 * N, 4], [2 * N, 2], [1, 64]]
                    nc.sync.dma_start(out=rr, in_=ap(src_dram, base, pat))
                    nc.sync.dma_start(out=ri, in_=ap(src_dram, base + N, pat))
                    pA_re = ps.tile([128, 512], F32, tag="ps_a_re")
                    pA_im = ps.tile([128, 512], F32, tag="ps_a_im")
                    nc.tensor.matmul(pA_re, cs["fa_re"], rr, start=True, stop=False)
                    nc.tensor.matmul(pA_re, cs["fa_imn"], ri, start=False, stop=True)
                    nc.tensor.matmul(pA_im, cs["fa_im"], rr, start=True, stop=False)
                    nc.tensor.matmul(pA_im, cs["fa_re"], ri, start=False, stop=True)
                    m1 = pc.tile([128, 512], BF16, tag=f"{pfx}_m1")
                    m2 = pc.tile([128, 512], BF16, tag=f"{pfx}_m2")
                    m3 = pc.tile([128, 512], BF16, tag=f"{pfx}_m3")
                    m4 = pc.tile([128, 512], BF16, tag=f"{pfx}_m4")
                    b_re = pc.tile([128, 512], BF16, tag=f"{pfx}_bre")
                    b_im = pc.tile([128, 512], BF16, tag=f"{pfx}_bim")
                    nc.vector.tensor_tensor(m1, pA_re, cs["tw_re"], ALU.mult)
                    nc.vector.tensor_tensor(m2, pA_im, cs["tw_im"], ALU.mult)
                    nc.gpsimd.tensor_tensor(b_re, m1, m2, ALU.subtract)
                    nc.vector.tensor_tensor(m3, pA_re, cs["tw_im"], ALU.mult)
                    nc.vector.tensor_tensor(m4, pA_im, cs["tw_re"], ALU.mult)
                    nc.gpsimd.tensor_tensor(b_im, m3, m4, ALU.add)
                    bt_re = pc.tile([128, 512], BF16, tag=f"{pfx}_btre")
                    bt_im = pc.tile([128, 512], BF16, tag=f"{pfx}_btim")
                    for tau in range(4):
                        for bsrc, bdst in ((b_re, bt_re), (b_im, bt_im)):
                            ptp = pst.tile([128, 128], BF16, tag="ps_tp")
                            nc.tensor.transpose(
                                ptp, bsrc[:, 128 * tau:128 * tau + 128],
                                cs["ident"])
                            nc.vector.tensor_copy(
                                out=bdst[:, 128 * tau:128 * tau + 128], in_=ptp)
                    return bt_re, bt_im

                def rev_rhs(bt):
                    """free-reversed read of Bt: cols (q, 128-k1'), k1' in [1,127]."""
                    po = bt.ap[0][0]
                    return ap(bt.tensor, bt.offset + 127,
                              [[po, 128], [128, 4], [-1, 127]])

                def col0_rhs(bt):
                    po = bt.ap[0][0]
                    return ap(bt.tensor, bt.offset, [[po, 128], [128, 4]])

                def stage_b(psum, lhs_main, lhs0, bt_re, bt_im, lhsn=None):
                    """psum[:, q*128 + k1'] = sum G-variant combos; main cols
                    k1'>=1 from reversed/straight rhs; col 0 via lhs0."""
                    # straight variant: lhs_main applied to straight rhs incl col0
                    raise NotImplementedError

                def mm_pair(psum, lA, rA, lB, rB, out_ap=None):
                    o = psum if out_ap is None else out_ap
                    nc.tensor.matmul(o, lA, rA, start=True, stop=False)
                    nc.tensor.matmul(o, lB, rB, start=False, stop=True)

                for q4 in range(NQUAD):
                    tbt_re, tbt_im = stage_a_and_transpose(t_bf, q4, "t")
                    vbt_re, vbt_im = stage_a_and_transpose(v_bf, q4, "v")

                    # ---- t-side stage B -> e1, f, e1r, fr directly
                    p_e1 = ps.tile([128, 512], F32, tag="ps_p1")
                    p_f = ps.tile([128, 512], F32, tag="ps_p2")
                    p_e1r = ps.tile([128, 512], F32, tag="ps_p3")
                    p_fr = ps.tile([128, 512], F32, tag="ps_p4")
                    mm_pair(p_e1, cs["ge0"], tbt_re, cs["gf0"], tbt_im)
                    mm_pair(p_f, cs["gf0"], tbt_re, cs["nge0"], tbt_im)
                    rre, rim = rev_rhs(tbt_re), rev_rhs(tbt_im)
                    c0re, c0im = col0_rhs(tbt_re), col0_rhs(tbt_im)
                    mm_pair(None, cs["ge1"], rre, cs["gf1"], rim,
                            out_ap=p_e1r[:, bass.ts(0, 512)].rearrange(
                                "p (q k) -> p q k", q=4)[:, :, 1:128])
                    mm_pair(None, cs["ge1c"], c0re, cs["gf1c"], c0im,
                            out_ap=p_e1r[:, bass.ts(0, 512)].rearrange(
                                "p (q k) -> p q k", q=4)[:, :, 0:1])
                    mm_pair(None, cs["gf1"], rre, cs["nge1"], rim,
                            out_ap=p_fr[:, bass.ts(0, 512)].rearrange(
                                "p (q k) -> p q k", q=4)[:, :, 1:128])
                    mm_pair(None, cs["gf1c"], c0re, cs["nge1c"], c0im,
                            out_ap=p_fr[:, bass.ts(0, 512)].rearrange(
                                "p (q k) -> p q k", q=4)[:, :, 0:1])
                    e1 = sp.tile([128, 512], BF16, tag="e1")
                    f_ = sp.tile([128, 512], BF16, tag="f_")
                    e1r = sp.tile([128, 512], BF16, tag="e1r")
                    fr = sp.tile([128, 512], BF16, tag="fr")
                    nc.scalar.activation(e1, p_e1, AF.Copy)
                    nc.scalar.activation(f_, p_f, AF.Copy)
                    nc.scalar.activation(e1r, p_e1r, AF.Copy)
                    nc.scalar.activation(fr, p_fr, AF.Copy)
                    s4re = sp.tile([128, 512], BF16, tag="s4re")
                    s4im = sp.tile([128, 512], BF16, tag="s4im")
                    d4re = sp.tile([128, 512], BF16, tag="d4re")
                    d4im = sp.tile([128, 512], BF16, tag="d4im")
                    nc.vector.tensor_tensor(s4re, e1, e1r, ALU.add)
                    nc.vector.tensor_tensor(s4im, fr, f_, ALU.subtract)
                    nc.gpsimd.tensor_tensor(d4re, f_, fr, ALU.add)
                    nc.gpsimd.tensor_tensor(d4im, e1, e1r, ALU.subtract)

                    # ---- v-side stage B -> Z and rev(Z)
                    p_zre = ps.tile([128, 512], F32, tag="ps_p1")
                    p_zim = ps.tile([128, 512], F32, tag="ps_p2")
                    mm_pair(p_zre, cs["g_re"], vbt_re, cs["g_imn"], vbt_im)
                    mm_pair(p_zim, cs["g_im"], vbt_re, cs["g_re"], vbt_im)
                    p_rre = ps.tile([128, 512], F32, tag="ps_p3")
                    p_rim = ps.tile([128, 512], F32, tag="ps_p4")
                    vrre, vrim = rev_rhs(vbt_re), rev_rhs(vbt_im)
                    vc0re, vc0im = col0_rhs(vbt_re), col0_rhs(vbt_im)
                    mm_pair(None, cs["grev_re"], vrre, cs["grev_imn"], vrim,
                            out_ap=p_rre[:, bass.ts(0, 512)].rearrange(
                                "p (q k) -> p q k", q=4)[:, :, 1:128])
                    mm_pair(None, cs["grev0_re"], vc0re, cs["grev0_imn"], vc0im,
                            out_ap=p_rre[:, bass.ts(0, 512)].rearrange(
                                "p (q k) -> p q k", q=4)[:, :, 0:1])
                    mm_pair(None, cs["grev_im"], vrre, cs["grev_re"], vrim,
                            out_ap=p_rim[:, bass.ts(0, 512)].rearrange(
                                "p (q k) -> p q k", q=4)[:, :, 1:128])
                    mm_pair(None, cs["grev0_im"], vc0re, cs["grev0_re"], vc0im,
                            out_ap=p_rim[:, bass.ts(0, 512)].rearrange(
                                "p (q k) -> p q k", q=4)[:, :, 0:1])
                    zv_re = sp.tile([128, 512], BF16, tag="zv_re")
                    zv_im = sp.tile([128, 512], BF16, tag="zv_im")
                    zr_re = sp.tile([128, 512], BF16, tag="zr_re")
                    zr_im = sp.tile([128, 512], BF16, tag="zr_im")
                    nc.scalar.activation(zv_re, p_zre, AF.Copy)
                    nc.scalar.activation(zv_im, p_zim, AF.Copy)
                    nc.scalar.activation(zr_re, p_rre, AF.Copy)
                    nc.scalar.activation(zr_im, p_rim, AF.Copy)

                    # ---- Q = Z*S4 + conj(rev Z)*D4
                    q_re = sp.tile([128, 512], BF16, tag="q_re")
                    q_im = sp.tile([128, 512], BF16, tag="q_im")
                    a1 = pc.tile([128, 512], BF16, tag="qa1")
                    a2 = pc.tile([128, 512], BF16, tag="qa2")
                    a3 = pc.tile([128, 512], BF16, tag="qa3")
                    nc.vector.tensor_tensor(a1, zv_re, s4re, ALU.mult)
                    nc.vector.tensor_tensor(a2, zv_im, s4im, ALU.mult)
                    nc.vector.tensor_tensor(a1, a1, a2, ALU.subtract)
                    nc.vector.tensor_tensor(a2, zr_re, d4re, ALU.mult)
                    nc.vector.tensor_tensor(a3, zr_im, d4im, ALU.mult)
                    nc.vector.tensor_tensor(a2, a2, a3, ALU.add)
                    nc.vector.tensor_tensor(q_re, a1, a2, ALU.add)
                    b1 = pc.tile([128, 512], BF16, tag="qb1")
                    b2 = pc.tile([128, 512], BF16, tag="qb2")
                    b3 = pc.tile([128, 512], BF16, tag="qb3")
                    nc.gpsimd.tensor_tensor(b1, zv_re, s4im, ALU.mult)
                    nc.gpsimd.tensor_tensor(b2, zv_im, s4re, ALU.mult)
                    nc.gpsimd.tensor_tensor(b1, b1, b2, ALU.add)
                    nc.gpsimd.tensor_tensor(b2, zr_re, d4im, ALU.mult)
                    nc.gpsimd.tensor_tensor(b3, zr_im, d4re, ALU.mult)
                    nc.gpsimd.tensor_tensor(b2, b2, b3, ALU.subtract)
                    nc.gpsimd.tensor_tensor(q_im, b1, b2, ALU.add)

                    # ---- inverse
                    pC_re = ps.tile([128, 512], F32, tag="ps_a_re")
                    pC_im = ps.tile([128, 512], F32, tag="ps_a_im")
                    mm_pair(pC_re, cs["gi_re"], q_re, cs["gi_imn"], q_im)
                    mm_pair(pC_im, cs["gi_im"], q_re, cs["gi_re"], q_im)
                    m1 = pc.tile([128, 512], BF16, tag="i_m1")
                    m2 = pc.tile([128, 512], BF16, tag="i_m2")
                    m3 = pc.tile([128, 512], BF16, tag="i_m3")
                    m4 = pc.tile([128, 512], BF16, tag="i_m4")
                    ct_re = pc.tile([128, 512], BF16, tag="ct_re")
                    ct_im = pc.tile([128, 512], BF16, tag="ct_im")
                    nc.vector.tensor_tensor(m1, pC_re, cs["ti_re"], ALU.mult)
                    nc.vector.tensor_tensor(m2, pC_im, cs["ti_im"], ALU.mult)
                    nc.gpsimd.tensor_tensor(ct_re, m1, m2, ALU.subtract)
                    nc.vector.tensor_tensor(m3, pC_re, cs["ti_im"], ALU.mult)
                    nc.vector.tensor_tensor(m4, pC_im, cs["ti_re"], ALU.mult)
                    nc.gpsimd.tensor_tensor(ct_im, m3, m4, ALU.add)
                    ctt_re = pc.tile([128, 512], BF16, tag="ctt_re")
                    ctt_im = pc.tile([128, 512], BF16, tag="ctt_im")
                    for tau in range(4):
                        for csrc, cdst in ((ct_re, ctt_re), (ct_im, ctt_im)):
                            ptp = pst.tile([128, 128], BF16, tag="ps_tp")
                            nc.tensor.transpose(
                                ptp, csrc[:, 128 * tau:128 * tau + 128],
                                cs["ident"])
                            nc.vector.tensor_copy(
                                out=cdst[:, 128 * tau:128 * tau + 128], in_=ptp)
                    pO_re = ps.tile([64, 512], F32, tag="ps_p1")
                    pO_im = ps.tile([64, 512], F32, tag="ps_p2")
                    mm_pair(pO_re, cs["fi_re"], ctt_re, cs["fi_imn"], ctt_im)
                    mm_pair(pO_im, cs["fi_im"], ctt_re, cs["fi_re"], ctt_im)
                    o_re = pc.tile([64, 512], BF16, tag="o_re")
                    o_im = pc.tile([64, 512], BF16, tag="o_im")
                    nc.scalar.activation(o_re, pO_re, AF.Copy)
                    nc.scalar.activation(o_im, pO_im, AF.Copy)
                    base = 16 * q4 * N
                    pat = [[64, 64], [4 * N, 4], [2 * N, 2], [1, 64]]
                    nc.sync.dma_start(out=ap(o_bf, base, pat), in_=o_re)
                    nc.sync.dma_start(out=ap(o_bf, base + N, pat), in_=o_im)

            # =====================================================
            # Phase D: gate + output projection (f32r partials)
            # =====================================================
            with tc.tile_pool(name="pd_w", bufs=1) as wdp, \
                 tc.tile_pool(name="pd", bufs=2) as pd, \
                 tc.tile_pool(name="pd_ps", bufs=2, space="PSUM") as dps:
                wo_sb = wdp.tile([128, 4, D], F32R, tag="wo_sb")
                nc.gpsimd.dma_start(
                    out=wo_sb, in_=ap(wo, 0, [[D, 128], [128 * D, 4], [1, D]]))
                for sb in range(8):
                    gts = []
                    for cb in range(4):
                        ut = pd.tile([128, 512], BF16, tag=f"g_u{cb}")
                        ot = pd.tile([128, 512], BF16, tag=f"g_o{cb}")
                        nc.sync.dma_start(
                            out=ut, in_=ap(u_bf, 128 * cb * N + 512 * sb,
                                           [[N, 128], [1, 512]]))
                        nc.sync.dma_start(
                            out=ot, in_=ap(o_bf, 128 * cb * N + 512 * sb,
                                           [[N, 128], [1, 512]]))
                        gt = pd.tile([128, 512], F32R, tag=f"g_g{cb}")
                        nc.vector.tensor_tensor(gt, ut, ot, ALU.mult)
                        gts.append(gt)
                    for ocblk in range(8):
                        po = dps.tile([128, 512], F32, tag="out_mm")
                        for cb in range(4):
                            nc.tensor.matmul(
                                po, wo_sb[:, cb, 128 * ocblk:128 * ocblk + 128],
                                gts[cb], start=(cb == 0), stop=(cb == 3))
                        os_ = pd.tile([128, 512], F32, tag="out_sb")
                        nc.scalar.activation(os_, po, AF.Copy)
                        nc.sync.dma_start(
                            out=ap(out, 128 * ocblk * N + 512 * sb,
                                   [[N, 128], [1, 512]]),
                            in_=os_)
    return nc


_PROGRAM_CACHE = {}


def _get_program():
    if "nc" not in _PROGRAM_CACHE:
        nc = bacc.Bacc("TRN2", target_bir_lowering=False)
        build_program(nc)
        nc.compile()
        _PROGRAM_CACHE["nc"] = nc
    return _PROGRAM_CACHE["nc"]


def kernel(x, W_uv, W_o, rpe_in_w, rpe_hid_w, rpe_ln_g, rpe_ln_b, rpe_out_w,
           decay_gamma):
    x = np.asarray(x, np.float32)
    W_uv = np.asarray(W_uv, np.float32)
    W_o = np.asarray(W_o, np.float32)

    nc = _get_program()

    shared = dict(CONSTS)
    shared["rpe_in"] = np.ascontiguousarray(rpe_in_w, np.float32)
    shared["rpe_hid"] = np.ascontiguousarray(rpe_hid_w, np.float32)
    shared["ln_g"] = np.ascontiguousarray(rpe_ln_g, np.float32)
    shared["ln_b"] = np.ascontiguousarray(rpe_ln_b, np.float32)

    in_maps = []
    for core in range(8):
        b, h = core // 2, core % 2
        c0 = h * H
        m = dict(shared)
        m["x"] = np.ascontiguousarray(x[b])
        m["wuv"] = np.ascontiguousarray(
            np.concatenate([W_uv[:, c0:c0 + H], W_uv[:, D1 + c0:D1 + c0 + H]],
                           axis=1))
        m["wo"] = np.ascontiguousarray(np.asarray(W_o, np.float32)[c0:c0 + H, :])
        m["rpeo"] = np.ascontiguousarray(np.asarray(rpe_out_w, np.float32)[:, c0:c0 + H])
        m["dg"] = np.ascontiguousarray(
            np.asarray(decay_gamma, np.float32)[None, c0:c0 + H])
        in_maps.append(m)

    import os
    kw = {}
    if os.environ.get("KERNEL_TRACE"):
        kw = dict(trace=True, tmpdir=os.environ.get("KERNEL_TRACE_DIR") or None)
    res = run_bass_kernel_spmd(nc, in_maps, core_ids=list(range(8)), **kw)
    global LAST_RESULTS
    LAST_RESULTS = res
    outs = [r["out"] for r in res.results]
    final = np.empty((B, N, D), np.float32)
    for b in range(B):
        final[b] = (outs[2 * b] + outs[2 * b + 1]).T
    return final



# revision 14
# speedup vs baseline: 1.1527x; 1.1527x over previous
"""Trainium2 Bass kernel for nn_Gtu (gated Toeplitz unit / TNN GTU layer).

  uv = silu(x @ W_uv); u, v = split(uv)
  t  = RPE-MLP(arange(n)) * gamma^k          (per-channel causal Toeplitz coefs)
  o  = causal_conv(t, v)                     (per channel, via length-8192 FFT)
  out = (u * o) @ W_o

8 cores = (batch 0..3) x (d1-half 0..1). Each core handles its batch and 512
channels end-to-end plus the partial output projection; the host sums the two
partials per batch (row-split of W_o) and concatenates batches.

FFT: four-step matmul factorization, L = 8192 = 128 x 64:
  n = n1*64 + n2 (n1 in [0,128) contracted; input support n1 < 64)
  k = k1 + 128*k2
  A[k1,(q,s,n2)] = sum_n1 in[n1] W128^(n1 k1)            (stage A, PE)
  B = A * W8192^(k1 n2)                                  (twiddle, DVE/GP)
  per-tau transpose [k1,(s,n2)] -> [(s,n2),k1]           (PE)
  Z[(s,k2),(q,k1)] = sum_n2 B^T W64^(n2 k2) (block-diag) (stage B, PE)
Real channels packed in pairs z = v_c + i*v_{c+1} (s in {0,1} per tile; a
"quad" = 4 tiles = 16 real channels, free dim 512). Spectral multiply:
  e1 = Wre+Wim, f = Wre-Wim (W = packed t-spectrum)
  S4 = (e1 + rev f) + i(rev f - f);  D4 = (f + rev e1) + i(e1 - rev e1)
  Q4 = Z*S4 + conj(rev Z)*D4        (= 4*(Zc Tc + i Zc1 Tc1) packed product)
Inverse mirrors forward; 1/(4L) is folded into the final inverse DFT matrix.
Re/Im of the inverse are o_c / o_{c+1}.
"""

import numpy as np
import ml_dtypes

import concourse.bass as bass
import concourse.tile as tile
import concourse.mybir as mybir
from concourse import bacc
from concourse.bass_utils import run_bass_kernel_spmd

F32 = mybir.dt.float32
F32R = mybir.dt.float32r
BF16 = mybir.dt.bfloat16
AF = mybir.ActivationFunctionType
ALU = mybir.AluOpType
AXX = mybir.AxisListType.X

B, N, D = 4, 4096, 1024
D1 = 1024
H = 512
L = 8192
FEAT = 32
RPE_LAYERS = 3
LOWER = 0.99
LN_EPS = 1e-5
NQUAD = 32

_NP_BF16 = ml_dtypes.bfloat16


def _host_consts():
    c = {}
    bf = lambda a: np.ascontiguousarray(a, dtype=_NP_BF16)
    f32 = lambda a: np.ascontiguousarray(a, dtype=np.float32)

    n1 = np.arange(64)[:, None]
    k1 = np.arange(128)[None, :]
    w = np.exp(-2j * np.pi * n1 * k1 / 128.0)
    c["fa_re"], c["fa_im"], c["fa_imn"] = bf(w.real), bf(w.imag), bf(-w.imag)

    k1c = np.arange(128)[:, None]
    n2c = np.arange(64)[None, :]
    tw = np.tile(np.exp(-2j * np.pi * k1c * n2c / float(L)), (1, 8))
    c["tw_re"], c["tw_im"] = bf(tw.real), bf(tw.imag)

    # v2 stage A: moving DFT matrices [n1=64, k1=128] and post-transpose
    # twiddle [(s,n2)=128, (q,k1)=512]
    c["wk_re"], c["wk_im"], c["wk_imn"] = bf(w.real), bf(w.imag), bf(-w.imag)
    sn = np.arange(128)[:, None] % 64            # n2 per (s,n2) row
    qk = np.tile(np.arange(128)[None, :], (1, 4))  # k1 per (q,k1) col
    twf = np.exp(-2j * np.pi * sn * qk / float(L))
    c["twf_re"], c["twf_im"] = bf(twf.real), bf(twf.imag)

    n2 = np.arange(64)[:, None]
    k2 = np.arange(64)[None, :]
    g = np.exp(-2j * np.pi * n2 * k2 / 64.0)
    gb = np.zeros((128, 128), np.complex128)
    gb[:64, :64] = g
    gb[64:, 64:] = g
    c["g_re"], c["g_im"], c["g_imn"] = bf(gb.real), bf(gb.imag), bf(-gb.imag)
    gi = np.conj(gb)
    c["gi_re"], c["gi_im"], c["gi_imn"] = bf(gi.real), bf(gi.imag), bf(-gi.imag)

    n2r = np.arange(64)[:, None]
    k1r = np.arange(128)[None, :]
    ti = np.exp(+2j * np.pi * n2r * k1r / float(L))
    tit = np.tile(np.concatenate([ti, ti], axis=0), (1, 4))
    c["ti_re"], c["ti_im"] = bf(tit.real), bf(tit.imag)

    k1f = np.arange(128)[:, None]
    n1f = np.arange(64)[None, :]
    fi = np.exp(+2j * np.pi * k1f * n1f / 128.0) / (4.0 * L)
    c["fi_re"], c["fi_im"], c["fi_imn"] = bf(fi.real), bf(fi.imag), bf(-fi.imag)

    # reversal stage-B variants: Zrev[k2'] uses G columns 63-k2' (main) and
    # (64-k2')%64 (the k1=0 column); t-side combos produce e1/f/e1r/fr directly
    def blockdiag(m):
        out = np.zeros((128, 128), np.complex128)
        out[:64, :64] = m
        out[64:, 64:] = m
        return out
    n2v = np.arange(64)[:, None]
    k2v = np.arange(64)[None, :]
    grev = blockdiag(np.exp(-2j * np.pi * n2v * (63 - k2v) / 64.0))
    grev0 = blockdiag(np.exp(-2j * np.pi * n2v * ((64 - k2v) % 64) / 64.0))
    c["grev_re"], c["grev_im"], c["grev_imn"] = bf(grev.real), bf(grev.imag), bf(-grev.imag)
    c["grev0_re"], c["grev0_im"], c["grev0_imn"] = bf(grev0.real), bf(grev0.imag), bf(-grev0.imag)
    c["ge0"] = bf(gb.real + gb.imag)
    c["gf0"] = bf(gb.real - gb.imag)
    c["nge0"] = bf(-(gb.real + gb.imag))
    c["ngf0"] = bf(-(gb.real - gb.imag))
    c["ge1"] = bf(grev.real + grev.imag)
    c["gf1"] = bf(grev.real - grev.imag)
    c["nge1"] = bf(-(grev.real + grev.imag))
    c["ngf1"] = bf(-(grev.real - grev.imag))
    c["ge1c"] = bf(grev0.real + grev0.imag)
    c["gf1c"] = bf(grev0.real - grev0.imag)
    c["nge1c"] = bf(-(grev0.real + grev0.imag))
    c["ngf1c"] = bf(-(grev0.real - grev0.imag))

    c["ident"] = bf(np.eye(128))
    c["ident_f32"] = f32(np.eye(128))

    p = np.arange(128)
    c["idxmat"] = f32(p[:, None] + 128.0 * np.arange(32)[None, :])
    c["pbc"] = f32(np.tile(p[None, :], (128, 1)))
    return c


CONSTS = _host_consts()


def build_program(nc):
    x = nc.dram_tensor("x", [N, D], F32, kind="ExternalInput")
    wuv = nc.dram_tensor("wuv", [D, 2 * H], F32, kind="ExternalInput")
    wo = nc.dram_tensor("wo", [H, D], F32, kind="ExternalInput")
    rpeo = nc.dram_tensor("rpeo", [FEAT, H], F32, kind="ExternalInput")
    dg = nc.dram_tensor("dg", [1, H], F32, kind="ExternalInput")
    rpe_in = nc.dram_tensor("rpe_in", [1, FEAT], F32, kind="ExternalInput")
    rpe_hid = nc.dram_tensor("rpe_hid", [RPE_LAYERS, FEAT, FEAT], F32,
                             kind="ExternalInput")
    ln_g = nc.dram_tensor("ln_g", [RPE_LAYERS, FEAT], F32, kind="ExternalInput")
    ln_b = nc.dram_tensor("ln_b", [RPE_LAYERS, FEAT], F32, kind="ExternalInput")

    cds = {}
    for name, arr in CONSTS.items():
        dt = BF16 if arr.dtype == _NP_BF16 else F32
        cds[name] = nc.dram_tensor(name, list(arr.shape), dt, kind="ExternalInput")

    t_bf = nc.dram_tensor("t_bf", [H, N], BF16, kind="Internal")
    v_bf = nc.dram_tensor("v_bf", [H, N], BF16, kind="Internal")
    u_bf = nc.dram_tensor("u_bf", [H, N], BF16, kind="Internal")
    o_bf = nc.dram_tensor("o_bf", [H, N], BF16, kind="Internal")
    lng_dram = nc.dram_tensor("lng_dram", [1, H], F32, kind="Internal")
    out = nc.dram_tensor("out", [D, N], F32, kind="ExternalOutput")

    ap = lambda t, off, pattern: bass.AP(tensor=t, offset=off, ap=pattern)

    with tile.TileContext(nc) as tc:
        with tc.tile_pool(name="consts", bufs=1) as cp:
            cs = {}
            for name, arr in CONSTS.items():
                dt = BF16 if arr.dtype == _NP_BF16 else F32
                ct = cp.tile(list(arr.shape), dt, tag=f"c_{name}")
                nc.sync.dma_start(out=ct, in_=cds[name][:, :])
                cs[name] = ct
            ident_r = cp.tile([128, 128], F32R, tag="ident_r")
            nc.gpsimd.dma_start(out=ident_r, in_=cds["ident_f32"][:, :])

            w_in_bc = cp.tile([128, FEAT], F32, tag="w_in_bc")
            nc.sync.dma_start(out=w_in_bc, in_=ap(rpe_in, 0, [[0, 128], [1, FEAT]]))
            lng_bc, lnb_bc = [], []
            for l in range(RPE_LAYERS):
                g_t = cp.tile([128, FEAT], F32, tag=f"lng{l}")
                b_t = cp.tile([128, FEAT], F32, tag=f"lnb{l}")
                nc.sync.dma_start(out=g_t, in_=ap(ln_g, l * FEAT, [[0, 128], [1, FEAT]]))
                nc.sync.dma_start(out=b_t, in_=ap(ln_b, l * FEAT, [[0, 128], [1, FEAT]]))
                lng_bc.append(g_t)
                lnb_bc.append(b_t)

            whid = []
            for l in range(RPE_LAYERS):
                wt4 = cp.tile([128, 128], F32, tag=f"whid{l}")
                nc.vector.memset(wt4, 0.0)
                for j in range(4):
                    nc.sync.dma_start(
                        out=wt4[32 * j:32 * j + 32, 32 * j:32 * j + 32],
                        in_=rpe_hid[l, :, :])
                whid.append(wt4)

            eps_t = cp.tile([128, 1], F32, tag="eps_t")
            nc.vector.memset(eps_t, LN_EPS)
            rpeo_sb = cp.tile([128, H], F32, tag="rpeo_sb")
            for j in range(4):
                nc.sync.dma_start(out=rpeo_sb[32 * j:32 * j + 32, :], in_=rpeo[:, :])

            # decay -> lngam_col [128, 4]  (lngam_col[cp, cb] = ln gamma_{128cb+cp})
            with tc.tile_pool(name="dk", bufs=1) as dk:
                dg_sb = dk.tile([1, H], F32, tag="dg")
                nc.sync.dma_start(out=dg_sb, in_=dg[:, :])
                sig = dk.tile([1, H], F32, tag="sig")
                nc.scalar.activation(sig, dg_sb, AF.Sigmoid)
                gam = dk.tile([1, H], F32, tag="gam")
                nc.vector.tensor_scalar(gam, sig, 1.0 - LOWER, LOWER,
                                        ALU.mult, ALU.add)
                lngr = dk.tile([1, H], F32, tag="lngr")
                nc.scalar.activation(lngr, gam, AF.Ln)
                nc.sync.dma_start(out=lng_dram[:, :], in_=lngr)
            lngam_col = cp.tile([128, 4], F32, tag="lngam_col")
            nc.sync.dma_start(out=lngam_col,
                              in_=ap(lng_dram, 0, [[1, 128], [128, 4]]))

            # =====================================================
            # Phase A: RPE MLP -> t_bf (channel-major bf16)
            # =====================================================
            idxm = cs["idxmat"]
            with tc.tile_pool(name="rpe", bufs=2) as rp, \
                 tc.tile_pool(name="rpe_ps", bufs=2, space="PSUM") as rps:
                for grp in range(8):
                    h_sm = rp.tile([128, 4, FEAT], F32, tag="h_sm")
                    for jj in range(4):
                        j = 4 * grp + jj
                        nc.scalar.activation(h_sm[:, jj, :], w_in_bc, AF.Silu,
                                             scale=idxm[:, j:j + 1])
                    h_fm = None
                    for l in range(RPE_LAYERS):
                        mu = rp.tile([128, 4], F32, tag="mu")
                        nc.vector.tensor_reduce(mu, h_sm, AXX, ALU.add)
                        nc.vector.tensor_scalar_mul(mu, mu, 1.0 / FEAT)
                        hc = rp.tile([128, 4, FEAT], F32, tag="hc")
                        nc.vector.tensor_tensor(
                            hc, h_sm, mu[:, :, None].to_broadcast((128, 4, FEAT)),
                            ALU.subtract)
                        sq = rp.tile([128, 4, FEAT], F32, tag="sq")
                        nc.scalar.activation(sq, hc, AF.Square)
                        var = rp.tile([128, 4], F32, tag="var")
                        nc.vector.tensor_reduce(var, sq, AXX, ALU.add)
                        rstd = rp.tile([128, 4], F32, tag="rstd")
                        nc.scalar.activation(rstd, var, AF.Sqrt,
                                             scale=1.0 / FEAT, bias=eps_t)
                        nc.vector.reciprocal(rstd, rstd)
                        hn = rp.tile([128, 4, FEAT], F32, tag="hn")
                        nc.vector.tensor_tensor(
                            hn, hc, rstd[:, :, None].to_broadcast((128, 4, FEAT)),
                            ALU.mult)
                        gb_ = lng_bc[l][:, None, :].to_broadcast((128, 4, FEAT))
                        bb_ = lnb_bc[l][:, None, :].to_broadcast((128, 4, FEAT))
                        hs = rp.tile([128, 4, FEAT], F32, tag="hs")
                        nc.vector.tensor_tensor(hs, hn, gb_, ALU.mult)
                        nc.vector.tensor_tensor(hs, hs, bb_, ALU.add)
                        pt = rps.tile([128, 128], F32, tag="tp")
                        nc.tensor.transpose(
                            pt, hs.rearrange("p a b -> p (a b)"), cs["ident_f32"])
                        ln_fm = rp.tile([128, 128], F32, tag="ln_fm")
                        nc.scalar.activation(ln_fm, pt, AF.Copy)
                        hp = rps.tile([128, 128], F32, tag="mm")
                        nc.tensor.matmul(hp, whid[l], ln_fm, start=True, stop=True)
                        h_fm = rp.tile([128, 128], F32, tag="h_fm")
                        nc.scalar.activation(h_fm, hp, AF.Silu)
                        if l < RPE_LAYERS - 1:
                            pt2 = rps.tile([128, 128], F32, tag="tp")
                            nc.tensor.transpose(pt2, h_fm, cs["ident_f32"])
                            nc.scalar.activation(
                                h_sm.rearrange("p a b -> p (a b)"), pt2, AF.Copy)
                    for jj in range(4):
                        j = 4 * grp + jj
                        for cb in range(4):
                            tp = rps.tile([128, 128], F32, tag="tmm")
                            nc.tensor.matmul(
                                tp,
                                rpeo_sb[32 * jj:32 * jj + 32,
                                        128 * cb:128 * cb + 128],
                                h_fm[32 * jj:32 * jj + 32, :],
                                start=True, stop=True,
                                tile_position=(32 * jj, 0))
                            ebias = rp.tile([128, 1], F32, tag="ebias")
                            nc.vector.tensor_scalar_mul(
                                ebias, lngam_col[:, cb:cb + 1], float(128 * j))
                            ee = rp.tile([128, 128], F32, tag="ee")
                            nc.scalar.activation(ee, cs["pbc"], AF.Exp,
                                                 scale=lngam_col[:, cb:cb + 1],
                                                 bias=ebias)
                            tt = rp.tile([128, 128], BF16, tag="t_out")
                            nc.vector.tensor_tensor(tt, tp, ee, ALU.mult)
                            nc.sync.dma_start(
                                out=ap(t_bf, 128 * cb * N + 128 * j,
                                       [[N, 128], [1, 128]]),
                                in_=tt)

            # =====================================================
            # Phase B: uv projection (f32r) + silu -> u_bf, v_bf
            # =====================================================
            with tc.tile_pool(name="pb_w", bufs=1) as wbp, \
                 tc.tile_pool(name="pb", bufs=2) as pb, \
                 tc.tile_pool(name="pb_ps", bufs=2, space="PSUM") as pps:
                wuv_sb = wbp.tile([128, 8, 2 * H], F32R, tag="wuv_sb")
                nc.gpsimd.dma_start(
                    out=wuv_sb,
                    in_=ap(wuv, 0, [[2 * H, 128], [128 * 2 * H, 8], [1, 2 * H]]))
                for stg in range(8):
                    xT = pb.tile([128, 8, 512], F32R, tag="xT")
                    for st4 in range(4):
                        st = 4 * stg + st4
                        xt = pb.tile([128, D], F32R, tag="x_in")
                        nc.gpsimd.dma_start(out=xt,
                                            in_=x[128 * st:128 * st + 128, :])
                        for k in range(8):
                            ptx = pps.tile([128, 128], F32R, tag="x_tp")
                            nc.tensor.transpose(
                                ptx, xt[:, 128 * k:128 * k + 128], ident_r)
                            nc.scalar.activation(
                                xT[:, k, 128 * st4:128 * st4 + 128], ptx, AF.Copy)
                    for cblk in range(8):
                        pu = pps.tile([128, 512], F32, tag="uv_mm")
                        for k in range(8):
                            nc.tensor.matmul(
                                pu, wuv_sb[:, k, 128 * cblk:128 * cblk + 128],
                                xT[:, k, :], start=(k == 0), stop=(k == 7))
                        uv_sb = pb.tile([128, 512], BF16, tag="uv_out")
                        nc.scalar.activation(uv_sb, pu, AF.Silu)
                        dst = u_bf if cblk < 4 else v_bf
                        cbase = (cblk % 4) * 128
                        nc.sync.dma_start(
                            out=ap(dst, cbase * N + 512 * stg,
                                   [[N, 128], [1, 512]]),
                            in_=uv_sb)

            # =====================================================
            # Phase C: FFT conv, one quad (16 channels) at a time
            # =====================================================
            with tc.tile_pool(name="pc", bufs=2) as pc, \
                 tc.tile_pool(name="pc_sp", bufs=2) as sp, \
                 tc.tile_pool(name="pc_ps", bufs=1, space="PSUM") as ps:

                def stage_a(src_dram, q4, pfx):
                    """DMA-gather + stationary-v stage A (output born
                    transposed) + post twiddle.
                    Returns (bt_re, bt_im) sbuf bf16 [(s,n2)=128,(q,k1)=512]."""
                    rr = pc.tile([64, 512], BF16, tag=f"{pfx}_rhs_re")
                    ri = pc.tile([64, 512], BF16, tag=f"{pfx}_rhs_im")
                    base = 16 * q4 * N
                    pat = [[64, 64], [4 * N, 4], [2 * N, 2], [1, 64]]
                    nc.sync.dma_start(out=rr, in_=ap(src_dram, base, pat))
                    nc.sync.dma_start(out=ri, in_=ap(src_dram, base + N, pat))
                    a_re = ps.tile([128, 512], F32, tag="ps_a_re")
                    a_im = ps.tile([128, 512], F32, tag="ps_a_im")
                    for q in range(4):
                        qs = slice(128 * q, 128 * q + 128)
                        rrq, riq = rr[:, qs], ri[:, qs]
                        nc.tensor.matmul(a_re[:, qs], rrq, cs["wk_re"],
                                         start=True, stop=False)
                        nc.tensor.matmul(a_im[:, qs], rrq, cs["wk_im"],
                                         start=True, stop=False)
                        nc.tensor.matmul(a_re[:, qs], riq, cs["wk_imn"],
                                         start=False, stop=True)
                        nc.tensor.matmul(a_im[:, qs], riq, cs["wk_re"],
                                         start=False, stop=True)
                    ca_re = pc.tile([128, 512], BF16, tag=f"{pfx}_ca_re")
                    ca_im = pc.tile([128, 512], BF16, tag=f"{pfx}_ca_im")
                    nc.scalar.activation(ca_re, a_re, AF.Copy)
                    nc.scalar.activation(ca_im, a_im, AF.Copy)
                    m1 = pc.tile([128, 512], BF16, tag="tm1")
                    m2 = pc.tile([128, 512], BF16, tag="tm2")
                    m3 = pc.tile([128, 512], BF16, tag="tm3")
                    m4 = pc.tile([128, 512], BF16, tag="tm4")
                    nc.vector.tensor_tensor(m1, ca_re, cs["twf_re"], ALU.mult)
                    nc.vector.tensor_tensor(m2, ca_im, cs["twf_im"], ALU.mult)
                    nc.vector.tensor_tensor(m3, ca_re, cs["twf_im"], ALU.mult)
                    nc.vector.tensor_tensor(m4, ca_im, cs["twf_re"], ALU.mult)
                    bt_re = pc.tile([128, 512], BF16, tag=f"{pfx}_btre")
                    bt_im = pc.tile([128, 512], BF16, tag=f"{pfx}_btim")
                    nc.gpsimd.tensor_tensor(bt_re, m1, m2, ALU.subtract)
                    nc.gpsimd.tensor_tensor(bt_im, m3, m4, ALU.add)
                    return bt_re, bt_im

                def rev_rhs(bt):
                    """free-reversed read of Bt: cols (q, 128-k1'), k1' in [1,127]."""
                    po = bt.ap[0][0]
                    return ap(bt.tensor, bt.offset + 127,
                              [[po, 128], [128, 4], [-1, 127]])

                def col0_rhs(bt):
                    po = bt.ap[0][0]
                    return ap(bt.tensor, bt.offset, [[po, 128], [128, 4]])

                def mm_pair(psum, lA, rA, lB, rB, out_ap=None):
                    o = psum if out_ap is None else out_ap
                    nc.tensor.matmul(o, lA, rA, start=True, stop=False)
                    nc.tensor.matmul(o, lB, rB, start=False, stop=True)

                for q4 in range(NQUAD):
                    tbt_re, tbt_im = stage_a(t_bf, q4, "t")
                    vbt_re, vbt_im = stage_a(v_bf, q4, "v")

                    # ---- t-side stage B -> s4re/s4im/d4re/d4im directly in
                    # PSUM via sign-folded G variants (linearity):
                    #   s4re = e1 + e1r;  s4im = fr - f
                    #   d4re = f + fr;    d4im = e1 - e1r
                    p_s4re = ps.tile([128, 512], F32, tag="ps_p1")
                    p_s4im = ps.tile([128, 512], F32, tag="ps_p2")
                    p_d4re = ps.tile([128, 512], F32, tag="ps_p3")
                    p_d4im = ps.tile([128, 512], F32, tag="ps_p4")
                    rre, rim = rev_rhs(tbt_re), rev_rhs(tbt_im)
                    c0re, c0im = col0_rhs(tbt_re), col0_rhs(tbt_im)
                    rng = lambda p, a, b: p[:, bass.ts(0, 512)].rearrange(
                        "p (q k) -> p q k", q=4)[:, :, a:b]

                    def acc4(p_out, lS1, lS2, lR1, lR2, lC1, lC2):
                        """p_out = lS1@bt_re + lS2@bt_im  (full cols)
                                 + lR1@rev_re + lR2@rev_im (cols 1..127)
                                 + lC1@c0_re + lC2@c0_im   (col 0)."""
                        nc.tensor.matmul(p_out, cs[lS1], tbt_re,
                                         start=True, stop=False)
                        nc.tensor.matmul(p_out, cs[lS2], tbt_im,
                                         start=False, stop=False)
                        nc.tensor.matmul(rng(p_out, 1, 128), cs[lR1], rre,
                                         start=False, stop=False)
                        nc.tensor.matmul(rng(p_out, 1, 128), cs[lR2], rim,
                                         start=False, stop=True)
                        nc.tensor.matmul(rng(p_out, 0, 1), cs[lC1], c0re,
                                         start=False, stop=False)
                        nc.tensor.matmul(rng(p_out, 0, 1), cs[lC2], c0im,
                                         start=False, stop=True)

                    acc4(p_s4re, "ge0", "gf0", "ge1", "gf1", "ge1c", "gf1c")
                    acc4(p_s4im, "ngf0", "ge0", "gf1", "nge1", "gf1c", "nge1c")
                    acc4(p_d4re, "gf0", "nge0", "gf1", "nge1", "gf1c", "nge1c")
                    acc4(p_d4im, "ge0", "gf0", "nge1", "ngf1", "nge1c", "ngf1c")
                    s4re = sp.tile([128, 512], BF16, tag="s4re")
                    s4im = sp.tile([128, 512], BF16, tag="s4im")
                    d4re = sp.tile([128, 512], BF16, tag="d4re")
                    d4im = sp.tile([128, 512], BF16, tag="d4im")
                    nc.scalar.activation(s4re, p_s4re, AF.Copy)
                    nc.scalar.activation(s4im, p_s4im, AF.Copy)
                    nc.scalar.activation(d4re, p_d4re, AF.Copy)
                    nc.scalar.activation(d4im, p_d4im, AF.Copy)

                    # ---- v-side stage B -> Z and rev(Z)
                    p_zre = ps.tile([128, 512], F32, tag="ps_p1")
                    p_zim = ps.tile([128, 512], F32, tag="ps_p2")
                    mm_pair(p_zre, cs["g_re"], vbt_re, cs["g_imn"], vbt_im)
                    mm_pair(p_zim, cs["g_im"], vbt_re, cs["g_re"], vbt_im)
                    p_rre = ps.tile([128, 512], F32, tag="ps_p3")
                    p_rim = ps.tile([128, 512], F32, tag="ps_p4")
                    vrre, vrim = rev_rhs(vbt_re), rev_rhs(vbt_im)
                    vc0re, vc0im = col0_rhs(vbt_re), col0_rhs(vbt_im)
                    mm_pair(None, cs["grev_re"], vrre, cs["grev_imn"], vrim,
                            out_ap=p_rre[:, bass.ts(0, 512)].rearrange(
                                "p (q k) -> p q k", q=4)[:, :, 1:128])
                    mm_pair(None, cs["grev0_re"], vc0re, cs["grev0_imn"], vc0im,
                            out_ap=p_rre[:, bass.ts(0, 512)].rearrange(
                                "p (q k) -> p q k", q=4)[:, :, 0:1])
                    mm_pair(None, cs["grev_im"], vrre, cs["grev_re"], vrim,
                            out_ap=p_rim[:, bass.ts(0, 512)].rearrange(
                                "p (q k) -> p q k", q=4)[:, :, 1:128])
                    mm_pair(None, cs["grev0_im"], vc0re, cs["grev0_re"], vc0im,
                            out_ap=p_rim[:, bass.ts(0, 512)].rearrange(
                                "p (q k) -> p q k", q=4)[:, :, 0:1])
                    zv_re = sp.tile([128, 512], BF16, tag="zv_re")
                    zv_im = sp.tile([128, 512], BF16, tag="zv_im")
                    zr_re = sp.tile([128, 512], BF16, tag="zr_re")
                    zr_im = sp.tile([128, 512], BF16, tag="zr_im")
                    nc.scalar.activation(zv_re, p_zre, AF.Copy)
                    nc.scalar.activation(zv_im, p_zim, AF.Copy)
                    nc.vector.tensor_copy(out=zr_re, in_=p_rre)
                    nc.vector.tensor_copy(out=zr_im, in_=p_rim)

                    # ---- Q = Z*S4 + conj(rev Z)*D4  (all DVE, bf16 4x)
                    q_re = sp.tile([128, 512], BF16, tag="q_re")
                    q_im = sp.tile([128, 512], BF16, tag="q_im")
                    a1 = pc.tile([128, 512], BF16, tag="qa1")
                    a2 = pc.tile([128, 512], BF16, tag="qa2")
                    a3 = pc.tile([128, 512], BF16, tag="qa3")
                    nc.vector.tensor_tensor(a1, zv_re, s4re, ALU.mult)
                    nc.vector.tensor_tensor(a2, zv_im, s4im, ALU.mult)
                    nc.vector.tensor_tensor(a1, a1, a2, ALU.subtract)
                    nc.vector.tensor_tensor(a2, zr_re, d4re, ALU.mult)
                    nc.vector.tensor_tensor(a3, zr_im, d4im, ALU.mult)
                    nc.vector.tensor_tensor(a2, a2, a3, ALU.add)
                    nc.vector.tensor_tensor(q_re, a1, a2, ALU.add)
                    b1 = pc.tile([128, 512], BF16, tag="qb1")
                    b2 = pc.tile([128, 512], BF16, tag="qb2")
                    b3 = pc.tile([128, 512], BF16, tag="qb3")
                    nc.vector.tensor_tensor(b1, zv_re, s4im, ALU.mult)
                    nc.vector.tensor_tensor(b2, zv_im, s4re, ALU.mult)
                    nc.vector.tensor_tensor(b1, b1, b2, ALU.add)
                    nc.vector.tensor_tensor(b2, zr_re, d4im, ALU.mult)
                    nc.vector.tensor_tensor(b3, zr_im, d4re, ALU.mult)
                    nc.vector.tensor_tensor(b2, b2, b3, ALU.subtract)
                    nc.vector.tensor_tensor(q_im, b1, b2, ALU.add)

                    # ---- inverse
                    pC_re = ps.tile([128, 512], F32, tag="ps_a_re")
                    pC_im = ps.tile([128, 512], F32, tag="ps_a_im")
                    mm_pair(pC_re, cs["gi_re"], q_re, cs["gi_imn"], q_im)
                    mm_pair(pC_im, cs["gi_im"], q_re, cs["gi_re"], q_im)
                    c0_re = pc.tile([128, 512], BF16, tag="c0_re")
                    c0_im = pc.tile([128, 512], BF16, tag="c0_im")
                    nc.scalar.activation(c0_re, pC_re, AF.Copy)
                    nc.scalar.activation(c0_im, pC_im, AF.Copy)
                    m1 = pc.tile([128, 512], BF16, tag="i_m1")
                    m2 = pc.tile([128, 512], BF16, tag="i_m2")
                    m3 = pc.tile([128, 512], BF16, tag="i_m3")
                    m4 = pc.tile([128, 512], BF16, tag="i_m4")
                    ct_re = pc.tile([128, 512], BF16, tag="ct_re")
                    ct_im = pc.tile([128, 512], BF16, tag="ct_im")
                    nc.vector.tensor_tensor(m1, c0_re, cs["ti_re"], ALU.mult)
                    nc.vector.tensor_tensor(m2, c0_im, cs["ti_im"], ALU.mult)
                    nc.vector.tensor_tensor(ct_re, m1, m2, ALU.subtract)
                    nc.vector.tensor_tensor(m3, c0_re, cs["ti_im"], ALU.mult)
                    nc.vector.tensor_tensor(m4, c0_im, cs["ti_re"], ALU.mult)
                    nc.vector.tensor_tensor(ct_im, m3, m4, ALU.add)
                    ctp_re = ps.tile([128, 512], BF16, tag="ps_ctt_re")
                    ctp_im = ps.tile([128, 512], BF16, tag="ps_ctt_im")
                    for tau in range(4):
                        ts_ = slice(128 * tau, 128 * tau + 128)
                        nc.tensor.transpose(ctp_re[:, ts_], ct_re[:, ts_],
                                            cs["ident"])
                        nc.tensor.transpose(ctp_im[:, ts_], ct_im[:, ts_],
                                            cs["ident"])
                    ctt_re = pc.tile([128, 512], BF16, tag="ctt_re")
                    ctt_im = pc.tile([128, 512], BF16, tag="ctt_im")
                    nc.vector.tensor_copy(out=ctt_re, in_=ctp_re)
                    nc.vector.tensor_copy(out=ctt_im, in_=ctp_im)
                    pO_re_t = ps.tile([128, 512], F32, tag="ps_a_re")
                    pO_im_t = ps.tile([128, 512], F32, tag="ps_a_im")
                    pO_re, pO_im = pO_re_t[:64, :], pO_im_t[:64, :]
                    mm_pair(pO_re, cs["fi_re"], ctt_re, cs["fi_imn"], ctt_im)
                    mm_pair(pO_im, cs["fi_im"], ctt_re, cs["fi_re"], ctt_im)
                    o_re = pc.tile([64, 512], BF16, tag="o_re")
                    o_im = pc.tile([64, 512], BF16, tag="o_im")
                    nc.scalar.activation(o_re, pO_re, AF.Copy)
                    nc.scalar.activation(o_im, pO_im, AF.Copy)
                    base = 16 * q4 * N
                    pat = [[64, 64], [4 * N, 4], [2 * N, 2], [1, 64]]
                    nc.sync.dma_start(out=ap(o_bf, base, pat), in_=o_re)
                    nc.sync.dma_start(out=ap(o_bf, base + N, pat), in_=o_im)

            # =====================================================
            # Phase D: gate + output projection (f32r partials)
            # =====================================================
            with tc.tile_pool(name="pd_w", bufs=1) as wdp, \
                 tc.tile_pool(name="pd", bufs=2) as pd, \
                 tc.tile_pool(name="pd_ps", bufs=2, space="PSUM") as dps:
                wo_sb = wdp.tile([128, 4, D], F32R, tag="wo_sb")
                nc.gpsimd.dma_start(
                    out=wo_sb, in_=ap(wo, 0, [[D, 128], [128 * D, 4], [1, D]]))
                for sb in range(8):
                    gts = []
                    for cb in range(4):
                        ut = pd.tile([128, 512], BF16, tag=f"g_u{cb}")
                        ot = pd.tile([128, 512], BF16, tag=f"g_o{cb}")
                        nc.sync.dma_start(
                            out=ut, in_=ap(u_bf, 128 * cb * N + 512 * sb,
                                           [[N, 128], [1, 512]]))
                        nc.sync.dma_start(
                            out=ot, in_=ap(o_bf, 128 * cb * N + 512 * sb,
                                           [[N, 128], [1, 512]]))
                        gt = pd.tile([128, 512], F32R, tag=f"g_g{cb}")
                        nc.vector.tensor_tensor(gt, ut, ot, ALU.mult)
                        gts.append(gt)
                    for ocblk in range(8):
                        po = dps.tile([128, 512], F32, tag="out_mm")
                        for cb in range(4):
                            nc.tensor.matmul(
                                po, wo_sb[:, cb, 128 * ocblk:128 * ocblk + 128],
                                gts[cb], start=(cb == 0), stop=(cb == 3))
                        os_ = pd.tile([128, 512], F32, tag="out_sb")
                        nc.scalar.activation(os_, po, AF.Copy)
                        nc.sync.dma_start(
                            out=ap(out, 128 * ocblk * N + 512 * sb,
                                   [[N, 128], [1, 512]]),
                            in_=os_)
    return nc


_PROGRAM_CACHE = {}


def _get_program():
    if "nc" not in _PROGRAM_CACHE:
        nc = bacc.Bacc("TRN2", target_bir_lowering=False)
        build_program(nc)
        nc.compile()
        _PROGRAM_CACHE["nc"] = nc
    return _PROGRAM_CACHE["nc"]


def kernel(x, W_uv, W_o, rpe_in_w, rpe_hid_w, rpe_ln_g, rpe_ln_b, rpe_out_w,
           decay_gamma):
    x = np.asarray(x, np.float32)
    W_uv = np.asarray(W_uv, np.float32)
    W_o = np.asarray(W_o, np.float32)

    nc = _get_program()

    shared = dict(CONSTS)
    shared["rpe_in"] = np.ascontiguousarray(rpe_in_w, np.float32)
    shared["rpe_hid"] = np.ascontiguousarray(rpe_hid_w, np.float32)
    shared["ln_g"] = np.ascontiguousarray(rpe_ln_g, np.float32)
    shared["ln_b"] = np.ascontiguousarray(rpe_ln_b, np.float32)

    in_maps = []
    for core in range(8):
        b, h = core // 2, core % 2
        c0 = h * H
        m = dict(shared)
        m["x"] = np.ascontiguousarray(x[b])
        m["wuv"] = np.ascontiguousarray(
            np.concatenate([W_uv[:, c0:c0 + H], W_uv[:, D1 + c0:D1 + c0 + H]],
                           axis=1))
        m["wo"] = np.ascontiguousarray(np.asarray(W_o, np.float32)[c0:c0 + H, :])
        m["rpeo"] = np.ascontiguousarray(np.asarray(rpe_out_w, np.float32)[:, c0:c0 + H])
        m["dg"] = np.ascontiguousarray(
            np.asarray(decay_gamma, np.float32)[None, c0:c0 + H])
        in_maps.append(m)

    import os
    kw = {}
    if os.environ.get("KERNEL_TRACE"):
        kw = dict(trace=True, tmpdir=os.environ.get("KERNEL_TRACE_DIR") or None)
    res = run_bass_kernel_spmd(nc, in_maps, core_ids=list(range(8)), **kw)
    global LAST_RESULTS
    LAST_RESULTS = res
    outs = [r["out"] for r in res.results]
    final = np.empty((B, N, D), np.float32)
    for b in range(B):
        final[b] = (outs[2 * b] + outs[2 * b + 1]).T
    return final



# revision 16
# speedup vs baseline: 1.3367x; 1.1596x over previous
"""Trainium2 Bass kernel for nn_Gtu (gated Toeplitz unit / TNN GTU layer).

  uv = silu(x @ W_uv); u, v = split(uv)
  t  = RPE-MLP(arange(n)) * gamma^k          (per-channel causal Toeplitz coefs)
  o  = causal_conv(t, v)                     (per channel, via length-8192 FFT)
  out = (u * o) @ W_o

8 cores = (batch 0..3) x (d1-half 0..1). Each core handles its batch and 512
channels end-to-end plus the partial output projection; the host sums the two
partials per batch (row-split of W_o) and concatenates batches.

FFT: four-step matmul factorization, L = 8192 = 128 x 64:
  n = n1*64 + n2 (n1 in [0,128) contracted; input support n1 < 64)
  k = k1 + 128*k2
  A[k1,(q,s,n2)] = sum_n1 in[n1] W128^(n1 k1)            (stage A, PE)
  B = A * W8192^(k1 n2)                                  (twiddle, DVE/GP)
  per-tau transpose [k1,(s,n2)] -> [(s,n2),k1]           (PE)
  Z[(s,k2),(q,k1)] = sum_n2 B^T W64^(n2 k2) (block-diag) (stage B, PE)
Real channels packed in pairs z = v_c + i*v_{c+1} (s in {0,1} per tile; a
"quad" = 4 tiles = 16 real channels, free dim 512). Spectral multiply:
  e1 = Wre+Wim, f = Wre-Wim (W = packed t-spectrum)
  S4 = (e1 + rev f) + i(rev f - f);  D4 = (f + rev e1) + i(e1 - rev e1)
  Q4 = Z*S4 + conj(rev Z)*D4        (= 4*(Zc Tc + i Zc1 Tc1) packed product)
Inverse mirrors forward; 1/(4L) is folded into the final inverse DFT matrix.
Re/Im of the inverse are o_c / o_{c+1}.
"""

import numpy as np
import ml_dtypes

import concourse.bass as bass
import concourse.tile as tile
import concourse.mybir as mybir
from concourse import bacc
from concourse.bass_utils import run_bass_kernel_spmd

F32 = mybir.dt.float32
F32R = mybir.dt.float32r
BF16 = mybir.dt.bfloat16
AF = mybir.ActivationFunctionType
ALU = mybir.AluOpType
AXX = mybir.AxisListType.X

B, N, D = 4, 4096, 1024
D1 = 1024
H = 512
L = 8192
FEAT = 32
RPE_LAYERS = 3
LOWER = 0.99
LN_EPS = 1e-5
NQUAD = 32

_NP_BF16 = ml_dtypes.bfloat16


def _host_consts():
    c = {}
    bf = lambda a: np.ascontiguousarray(a, dtype=_NP_BF16)
    f32 = lambda a: np.ascontiguousarray(a, dtype=np.float32)

    n1 = np.arange(64)[:, None]
    k1 = np.arange(128)[None, :]
    w = np.exp(-2j * np.pi * n1 * k1 / 128.0)
    c["fa_re"], c["fa_im"], c["fa_imn"] = bf(w.real), bf(w.imag), bf(-w.imag)

    k1c = np.arange(128)[:, None]
    n2c = np.arange(64)[None, :]
    tw = np.tile(np.exp(-2j * np.pi * k1c * n2c / float(L)), (1, 8))
    c["tw_re"], c["tw_im"] = bf(tw.real), bf(tw.imag)

    # v2 stage A: moving DFT matrices [n1=64, k1=128] and post-transpose
    # twiddle [(s,n2)=128, (q,k1)=512]
    c["wk_re"], c["wk_im"], c["wk_imn"] = bf(w.real), bf(w.imag), bf(-w.imag)
    sn = np.arange(128)[:, None] % 64            # n2 per (s,n2) row
    qk = np.tile(np.arange(128)[None, :], (1, 4))  # k1 per (q,k1) col
    twf = np.exp(-2j * np.pi * sn * qk / float(L))
    c["twf_re"], c["twf_im"] = bf(twf.real), bf(twf.imag)

    n2 = np.arange(64)[:, None]
    k2 = np.arange(64)[None, :]
    g = np.exp(-2j * np.pi * n2 * k2 / 64.0)
    gb = np.zeros((128, 128), np.complex128)
    gb[:64, :64] = g
    gb[64:, 64:] = g
    c["g_re"], c["g_im"], c["g_imn"] = bf(gb.real), bf(gb.imag), bf(-gb.imag)
    gi = np.conj(gb)
    c["gi_re"], c["gi_im"], c["gi_imn"] = bf(gi.real), bf(gi.imag), bf(-gi.imag)

    n2r = np.arange(64)[:, None]
    k1r = np.arange(128)[None, :]
    ti = np.exp(+2j * np.pi * n2r * k1r / float(L))
    tit = np.tile(np.concatenate([ti, ti], axis=0), (1, 4))
    c["ti_re"], c["ti_im"] = bf(tit.real), bf(tit.imag)

    k1f = np.arange(128)[:, None]
    n1f = np.arange(64)[None, :]
    fi = np.exp(+2j * np.pi * k1f * n1f / 128.0) / (4.0 * L)
    c["fi_re"], c["fi_im"], c["fi_imn"] = bf(fi.real), bf(fi.imag), bf(-fi.imag)

    # reversal stage-B variants: Zrev[k2'] uses G columns 63-k2' (main) and
    # (64-k2')%64 (the k1=0 column); t-side combos produce e1/f/e1r/fr directly
    def blockdiag(m):
        out = np.zeros((128, 128), np.complex128)
        out[:64, :64] = m
        out[64:, 64:] = m
        return out
    n2v = np.arange(64)[:, None]
    k2v = np.arange(64)[None, :]
    grev = blockdiag(np.exp(-2j * np.pi * n2v * (63 - k2v) / 64.0))
    grev0 = blockdiag(np.exp(-2j * np.pi * n2v * ((64 - k2v) % 64) / 64.0))
    c["grev_re"], c["grev_im"], c["grev_imn"] = bf(grev.real), bf(grev.imag), bf(-grev.imag)
    c["grev0_re"], c["grev0_im"], c["grev0_imn"] = bf(grev0.real), bf(grev0.imag), bf(-grev0.imag)
    c["ge0"] = bf(gb.real + gb.imag)
    c["gf0"] = bf(gb.real - gb.imag)
    c["nge0"] = bf(-(gb.real + gb.imag))
    c["ngf0"] = bf(-(gb.real - gb.imag))
    c["ge1"] = bf(grev.real + grev.imag)
    c["gf1"] = bf(grev.real - grev.imag)
    c["nge1"] = bf(-(grev.real + grev.imag))
    c["ngf1"] = bf(-(grev.real - grev.imag))
    c["ge1c"] = bf(grev0.real + grev0.imag)
    c["gf1c"] = bf(grev0.real - grev0.imag)
    c["nge1c"] = bf(-(grev0.real + grev0.imag))
    c["ngf1c"] = bf(-(grev0.real - grev0.imag))

    c["ident"] = bf(np.eye(128))
    c["ident_f32"] = f32(np.eye(128))

    p = np.arange(128)
    c["idxmat"] = f32(p[:, None] + 128.0 * np.arange(32)[None, :])
    c["pbc"] = f32(np.tile(p[None, :], (128, 1)))
    return c


CONSTS = _host_consts()


def build_program(nc):
    x = nc.dram_tensor("x", [N, D], F32, kind="ExternalInput")
    wuv = nc.dram_tensor("wuv", [D, 2 * H], F32, kind="ExternalInput")
    wo = nc.dram_tensor("wo", [H, D], F32, kind="ExternalInput")
    rpeo = nc.dram_tensor("rpeo", [FEAT, H], F32, kind="ExternalInput")
    dg = nc.dram_tensor("dg", [1, H], F32, kind="ExternalInput")
    rpe_in = nc.dram_tensor("rpe_in", [1, FEAT], F32, kind="ExternalInput")
    rpe_hid = nc.dram_tensor("rpe_hid", [RPE_LAYERS, FEAT, FEAT], F32,
                             kind="ExternalInput")
    ln_g = nc.dram_tensor("ln_g", [RPE_LAYERS, FEAT], F32, kind="ExternalInput")
    ln_b = nc.dram_tensor("ln_b", [RPE_LAYERS, FEAT], F32, kind="ExternalInput")

    cds = {}
    for name, arr in CONSTS.items():
        dt = BF16 if arr.dtype == _NP_BF16 else F32
        cds[name] = nc.dram_tensor(name, list(arr.shape), dt, kind="ExternalInput")

    t_bf = nc.dram_tensor("t_bf", [H, N], BF16, kind="Internal")
    v_bf = nc.dram_tensor("v_bf", [H, N], BF16, kind="Internal")
    u_bf = nc.dram_tensor("u_bf", [H, N], BF16, kind="Internal")
    o_bf = nc.dram_tensor("o_bf", [H, N], BF16, kind="Internal")
    lng_dram = nc.dram_tensor("lng_dram", [1, H], F32, kind="Internal")
    out = nc.dram_tensor("out", [D, N], F32, kind="ExternalOutput")

    ap = lambda t, off, pattern: bass.AP(tensor=t, offset=off, ap=pattern)

    with tile.TileContext(nc) as tc:
        with tc.tile_pool(name="consts", bufs=1) as cp:
            cs = {}
            for name, arr in CONSTS.items():
                dt = BF16 if arr.dtype == _NP_BF16 else F32
                ct = cp.tile(list(arr.shape), dt, tag=f"c_{name}")
                nc.sync.dma_start(out=ct, in_=cds[name][:, :])
                cs[name] = ct
            ident_r = cp.tile([128, 128], F32R, tag="ident_r")
            nc.gpsimd.dma_start(out=ident_r, in_=cds["ident_f32"][:, :])

            w_in_bc = cp.tile([128, FEAT], F32, tag="w_in_bc")
            nc.sync.dma_start(out=w_in_bc, in_=ap(rpe_in, 0, [[0, 128], [1, FEAT]]))
            lng_bc, lnb_bc = [], []
            for l in range(RPE_LAYERS):
                g_t = cp.tile([128, FEAT], F32, tag=f"lng{l}")
                b_t = cp.tile([128, FEAT], F32, tag=f"lnb{l}")
                nc.sync.dma_start(out=g_t, in_=ap(ln_g, l * FEAT, [[0, 128], [1, FEAT]]))
                nc.sync.dma_start(out=b_t, in_=ap(ln_b, l * FEAT, [[0, 128], [1, FEAT]]))
                lng_bc.append(g_t)
                lnb_bc.append(b_t)

            whid = []
            for l in range(RPE_LAYERS):
                wt4 = cp.tile([128, 128], F32, tag=f"whid{l}")
                nc.vector.memset(wt4, 0.0)
                for j in range(4):
                    nc.sync.dma_start(
                        out=wt4[32 * j:32 * j + 32, 32 * j:32 * j + 32],
                        in_=rpe_hid[l, :, :])
                whid.append(wt4)

            eps_t = cp.tile([128, 1], F32, tag="eps_t")
            nc.vector.memset(eps_t, LN_EPS)
            rpeo_sb = cp.tile([128, H], F32, tag="rpeo_sb")
            for j in range(4):
                nc.sync.dma_start(out=rpeo_sb[32 * j:32 * j + 32, :], in_=rpeo[:, :])

            # decay -> lngam_col [128, 4]  (lngam_col[cp, cb] = ln gamma_{128cb+cp})
            with tc.tile_pool(name="dk", bufs=1) as dk:
                dg_sb = dk.tile([1, H], F32, tag="dg")
                nc.sync.dma_start(out=dg_sb, in_=dg[:, :])
                sig = dk.tile([1, H], F32, tag="sig")
                nc.scalar.activation(sig, dg_sb, AF.Sigmoid)
                gam = dk.tile([1, H], F32, tag="gam")
                nc.vector.tensor_scalar(gam, sig, 1.0 - LOWER, LOWER,
                                        ALU.mult, ALU.add)
                lngr = dk.tile([1, H], F32, tag="lngr")
                nc.scalar.activation(lngr, gam, AF.Ln)
                nc.sync.dma_start(out=lng_dram[:, :], in_=lngr)
            lngam_col = cp.tile([128, 4], F32, tag="lngam_col")
            nc.sync.dma_start(out=lngam_col,
                              in_=ap(lng_dram, 0, [[1, 128], [128, 4]]))

            # =====================================================
            # Phase A: RPE MLP -> t_bf (channel-major bf16)
            # =====================================================
            idxm = cs["idxmat"]
            with tc.tile_pool(name="rpe", bufs=2) as rp, \
                 tc.tile_pool(name="rpe_ps", bufs=2, space="PSUM") as rps:
                for grp in range(8):
                    h_sm = rp.tile([128, 4, FEAT], F32, tag="h_sm")
                    for jj in range(4):
                        j = 4 * grp + jj
                        nc.scalar.activation(h_sm[:, jj, :], w_in_bc, AF.Silu,
                                             scale=idxm[:, j:j + 1])
                    h_fm = None
                    for l in range(RPE_LAYERS):
                        mu = rp.tile([128, 4], F32, tag="mu")
                        nc.vector.tensor_reduce(mu, h_sm, AXX, ALU.add)
                        nc.vector.tensor_scalar_mul(mu, mu, 1.0 / FEAT)
                        hc = rp.tile([128, 4, FEAT], F32, tag="hc")
                        nc.vector.tensor_tensor(
                            hc, h_sm, mu[:, :, None].to_broadcast((128, 4, FEAT)),
                            ALU.subtract)
                        sq = rp.tile([128, 4, FEAT], F32, tag="sq")
                        nc.scalar.activation(sq, hc, AF.Square)
                        var = rp.tile([128, 4], F32, tag="var")
                        nc.vector.tensor_reduce(var, sq, AXX, ALU.add)
                        rstd = rp.tile([128, 4], F32, tag="rstd")
                        nc.scalar.activation(rstd, var, AF.Sqrt,
                                             scale=1.0 / FEAT, bias=eps_t)
                        nc.vector.reciprocal(rstd, rstd)
                        hn = rp.tile([128, 4, FEAT], F32, tag="hn")
                        nc.vector.tensor_tensor(
                            hn, hc, rstd[:, :, None].to_broadcast((128, 4, FEAT)),
                            ALU.mult)
                        gb_ = lng_bc[l][:, None, :].to_broadcast((128, 4, FEAT))
                        bb_ = lnb_bc[l][:, None, :].to_broadcast((128, 4, FEAT))
                        hs = rp.tile([128, 4, FEAT], F32, tag="hs")
                        nc.vector.tensor_tensor(hs, hn, gb_, ALU.mult)
                        nc.vector.tensor_tensor(hs, hs, bb_, ALU.add)
                        pt = rps.tile([128, 128], F32, tag="tp")
                        nc.tensor.transpose(
                            pt, hs.rearrange("p a b -> p (a b)"), cs["ident_f32"])
                        ln_fm = rp.tile([128, 128], F32, tag="ln_fm")
                        nc.scalar.activation(ln_fm, pt, AF.Copy)
                        hp = rps.tile([128, 128], F32, tag="mm")
                        nc.tensor.matmul(hp, whid[l], ln_fm, start=True, stop=True)
                        h_fm = rp.tile([128, 128], F32, tag="h_fm")
                        nc.scalar.activation(h_fm, hp, AF.Silu)
                        if l < RPE_LAYERS - 1:
                            pt2 = rps.tile([128, 128], F32, tag="tp")
                            nc.tensor.transpose(pt2, h_fm, cs["ident_f32"])
                            nc.scalar.activation(
                                h_sm.rearrange("p a b -> p (a b)"), pt2, AF.Copy)
                    for jj in range(4):
                        j = 4 * grp + jj
                        for cb in range(4):
                            tp = rps.tile([128, 128], F32, tag="tmm")
                            nc.tensor.matmul(
                                tp,
                                rpeo_sb[32 * jj:32 * jj + 32,
                                        128 * cb:128 * cb + 128],
                                h_fm[32 * jj:32 * jj + 32, :],
                                start=True, stop=True,
                                tile_position=(32 * jj, 0))
                            ebias = rp.tile([128, 1], F32, tag="ebias")
                            nc.vector.tensor_scalar_mul(
                                ebias, lngam_col[:, cb:cb + 1], float(128 * j))
                            ee = rp.tile([128, 128], F32, tag="ee")
                            nc.scalar.activation(ee, cs["pbc"], AF.Exp,
                                                 scale=lngam_col[:, cb:cb + 1],
                                                 bias=ebias)
                            tt = rp.tile([128, 128], BF16, tag="t_out")
                            nc.vector.tensor_tensor(tt, tp, ee, ALU.mult)
                            nc.sync.dma_start(
                                out=ap(t_bf, 128 * cb * N + 128 * j,
                                       [[N, 128], [1, 128]]),
                                in_=tt)

            # =====================================================
            # Phase B: uv projection (f32r) + silu -> u_bf, v_bf
            # =====================================================
            with tc.tile_pool(name="pb_w", bufs=1) as wbp, \
                 tc.tile_pool(name="pb", bufs=2) as pb, \
                 tc.tile_pool(name="pb_ps", bufs=2, space="PSUM") as pps:
                wuv_sb = wbp.tile([128, 8, 2 * H], F32R, tag="wuv_sb")
                nc.gpsimd.dma_start(
                    out=wuv_sb,
                    in_=ap(wuv, 0, [[2 * H, 128], [128 * 2 * H, 8], [1, 2 * H]]))
                for stg in range(8):
                    xT = pb.tile([128, 8, 512], F32R, tag="xT")
                    for st4 in range(4):
                        st = 4 * stg + st4
                        xt = pb.tile([128, D], F32R, tag="x_in")
                        nc.gpsimd.dma_start(out=xt,
                                            in_=x[128 * st:128 * st + 128, :])
                        for k in range(8):
                            ptx = pps.tile([128, 128], F32R, tag="x_tp")
                            nc.tensor.transpose(
                                ptx, xt[:, 128 * k:128 * k + 128], ident_r)
                            nc.scalar.activation(
                                xT[:, k, 128 * st4:128 * st4 + 128], ptx, AF.Copy)
                    for cblk in range(8):
                        pu = pps.tile([128, 512], F32, tag="uv_mm")
                        for k in range(8):
                            nc.tensor.matmul(
                                pu, wuv_sb[:, k, 128 * cblk:128 * cblk + 128],
                                xT[:, k, :], start=(k == 0), stop=(k == 7))
                        uv_sb = pb.tile([128, 512], BF16, tag="uv_out")
                        nc.scalar.activation(uv_sb, pu, AF.Silu)
                        dst = u_bf if cblk < 4 else v_bf
                        cbase = (cblk % 4) * 128
                        nc.sync.dma_start(
                            out=ap(dst, cbase * N + 512 * stg,
                                   [[N, 128], [1, 512]]),
                            in_=uv_sb)

            # =====================================================
            # Phase C: FFT conv, one quad (16 channels) at a time
            # =====================================================
            with tc.tile_pool(name="pc", bufs=2) as pc, \
                 tc.tile_pool(name="pc_sp", bufs=2) as sp, \
                 tc.tile_pool(name="pc_ps", bufs=1, space="PSUM") as ps:

                def stage_a(src_dram, q4, pfx):
                    """DMA-gather + stationary-v stage A (output born
                    transposed) + post twiddle.
                    Returns (bt_re, bt_im) sbuf bf16 [(s,n2)=128,(q,k1)=512]."""
                    rr = pc.tile([64, 512], BF16, tag=f"{pfx}_rhs_re")
                    ri = pc.tile([64, 512], BF16, tag=f"{pfx}_rhs_im")
                    base = 16 * q4 * N
                    pat = [[64, 64], [4 * N, 4], [2 * N, 2], [1, 64]]
                    nc.sync.dma_start(out=rr, in_=ap(src_dram, base, pat))
                    nc.sync.dma_start(out=ri, in_=ap(src_dram, base + N, pat))
                    a_re = ps.tile([128, 512], F32, tag="ps_a_re")
                    a_im = ps.tile([128, 512], F32, tag="ps_a_im")
                    for q in range(4):
                        qs = slice(128 * q, 128 * q + 128)
                        rrq, riq = rr[:, qs], ri[:, qs]
                        nc.tensor.matmul(a_re[:, qs], rrq, cs["wk_re"],
                                         start=True, stop=False)
                        nc.tensor.matmul(a_im[:, qs], rrq, cs["wk_im"],
                                         start=True, stop=False)
                        nc.tensor.matmul(a_re[:, qs], riq, cs["wk_imn"],
                                         start=False, stop=True)
                        nc.tensor.matmul(a_im[:, qs], riq, cs["wk_re"],
                                         start=False, stop=True)
                    ca_re = pc.tile([128, 512], BF16, tag=f"{pfx}_ca_re")
                    ca_im = pc.tile([128, 512], BF16, tag=f"{pfx}_ca_im")
                    nc.scalar.activation(ca_re, a_re, AF.Copy)
                    nc.scalar.activation(ca_im, a_im, AF.Copy)
                    m1 = pc.tile([128, 512], BF16, tag="tm1")
                    m2 = pc.tile([128, 512], BF16, tag="tm2")
                    m3 = pc.tile([128, 512], BF16, tag="tm3")
                    m4 = pc.tile([128, 512], BF16, tag="tm4")
                    nc.vector.tensor_tensor(m1, ca_re, cs["twf_re"], ALU.mult)
                    nc.vector.tensor_tensor(m2, ca_im, cs["twf_im"], ALU.mult)
                    nc.vector.tensor_tensor(m3, ca_re, cs["twf_im"], ALU.mult)
                    nc.vector.tensor_tensor(m4, ca_im, cs["twf_re"], ALU.mult)
                    bt_re = pc.tile([128, 512], BF16, tag=f"{pfx}_btre")
                    bt_im = pc.tile([128, 512], BF16, tag=f"{pfx}_btim")
                    nc.gpsimd.tensor_tensor(bt_re, m1, m2, ALU.subtract)
                    nc.gpsimd.tensor_tensor(bt_im, m3, m4, ALU.add)
                    return bt_re, bt_im

                def rev_rhs(bt):
                    """free-reversed read of Bt: cols (q, 128-k1'), k1' in [1,127]."""
                    po = bt.ap[0][0]
                    return ap(bt.tensor, bt.offset + 127,
                              [[po, 128], [128, 4], [-1, 127]])

                def col0_rhs(bt):
                    po = bt.ap[0][0]
                    return ap(bt.tensor, bt.offset, [[po, 128], [128, 4]])

                def mm_pair(psum, lA, rA, lB, rB, out_ap=None):
                    o = psum if out_ap is None else out_ap
                    nc.tensor.matmul(o, lA, rA, start=True, stop=False)
                    nc.tensor.matmul(o, lB, rB, start=False, stop=True)

                for q4 in range(NQUAD):
                    tbt_re, tbt_im = stage_a(t_bf, q4, "t")
                    vbt_re, vbt_im = stage_a(v_bf, q4, "v")

                    # ---- t-side stage B -> s4re/s4im/d4re/d4im directly in
                    # PSUM via sign-folded G variants (linearity):
                    #   s4re = e1 + e1r;  s4im = fr - f
                    #   d4re = f + fr;    d4im = e1 - e1r
                    p_s4re = ps.tile([128, 512], F32, tag="ps_p1")
                    p_s4im = ps.tile([128, 512], F32, tag="ps_p2")
                    p_d4re = ps.tile([128, 512], F32, tag="ps_p3")
                    p_d4im = ps.tile([128, 512], F32, tag="ps_p4")
                    rre, rim = rev_rhs(tbt_re), rev_rhs(tbt_im)
                    c0re, c0im = col0_rhs(tbt_re), col0_rhs(tbt_im)
                    rng = lambda p, a, b: p[:, bass.ts(0, 512)].rearrange(
                        "p (q k) -> p q k", q=4)[:, :, a:b]

                    def acc4(p_out, lS1, lS2, lR1, lR2, lC1, lC2):
                        """p_out = lS1@bt_re + lS2@bt_im  (full cols)
                                 + lR1@rev_re + lR2@rev_im (cols 1..127)
                                 + lC1@c0_re + lC2@c0_im   (col 0)."""
                        nc.tensor.matmul(p_out, cs[lS1], tbt_re,
                                         start=True, stop=False)
                        nc.tensor.matmul(p_out, cs[lS2], tbt_im,
                                         start=False, stop=False)
                        nc.tensor.matmul(rng(p_out, 1, 128), cs[lR1], rre,
                                         start=False, stop=False)
                        nc.tensor.matmul(rng(p_out, 1, 128), cs[lR2], rim,
                                         start=False, stop=True)
                        nc.tensor.matmul(rng(p_out, 0, 1), cs[lC1], c0re,
                                         start=False, stop=False)
                        nc.tensor.matmul(rng(p_out, 0, 1), cs[lC2], c0im,
                                         start=False, stop=True)

                    acc4(p_s4re, "ge0", "gf0", "ge1", "gf1", "ge1c", "gf1c")
                    acc4(p_s4im, "ngf0", "ge0", "gf1", "nge1", "gf1c", "nge1c")
                    acc4(p_d4re, "gf0", "nge0", "gf1", "nge1", "gf1c", "nge1c")
                    acc4(p_d4im, "ge0", "gf0", "nge1", "ngf1", "nge1c", "ngf1c")
                    s4re = sp.tile([128, 512], BF16, tag="s4re")
                    s4im = sp.tile([128, 512], BF16, tag="s4im")
                    d4re = sp.tile([128, 512], BF16, tag="d4re")
                    d4im = sp.tile([128, 512], BF16, tag="d4im")
                    nc.scalar.activation(s4re, p_s4re, AF.Copy)
                    nc.scalar.activation(s4im, p_s4im, AF.Copy)
                    nc.scalar.activation(d4re, p_d4re, AF.Copy)
                    nc.scalar.activation(d4im, p_d4im, AF.Copy)

                    # ---- v-side stage B -> Z and rev(Z)
                    p_zre = ps.tile([128, 512], F32, tag="ps_p1")
                    p_zim = ps.tile([128, 512], F32, tag="ps_p2")
                    mm_pair(p_zre, cs["g_re"], vbt_re, cs["g_imn"], vbt_im)
                    mm_pair(p_zim, cs["g_im"], vbt_re, cs["g_re"], vbt_im)
                    p_rre = ps.tile([128, 512], F32, tag="ps_p3")
                    p_rim = ps.tile([128, 512], F32, tag="ps_p4")
                    vrre, vrim = rev_rhs(vbt_re), rev_rhs(vbt_im)
                    vc0re, vc0im = col0_rhs(vbt_re), col0_rhs(vbt_im)
                    mm_pair(None, cs["grev_re"], vrre, cs["grev_imn"], vrim,
                            out_ap=p_rre[:, bass.ts(0, 512)].rearrange(
                                "p (q k) -> p q k", q=4)[:, :, 1:128])
                    mm_pair(None, cs["grev0_re"], vc0re, cs["grev0_imn"], vc0im,
                            out_ap=p_rre[:, bass.ts(0, 512)].rearrange(
                                "p (q k) -> p q k", q=4)[:, :, 0:1])
                    mm_pair(None, cs["grev_im"], vrre, cs["grev_re"], vrim,
                            out_ap=p_rim[:, bass.ts(0, 512)].rearrange(
                                "p (q k) -> p q k", q=4)[:, :, 1:128])
                    mm_pair(None, cs["grev0_im"], vc0re, cs["grev0_re"], vc0im,
                            out_ap=p_rim[:, bass.ts(0, 512)].rearrange(
                                "p (q k) -> p q k", q=4)[:, :, 0:1])
                    zv_re = sp.tile([128, 512], BF16, tag="zv_re")
                    zv_im = sp.tile([128, 512], BF16, tag="zv_im")
                    zr_re = sp.tile([128, 512], BF16, tag="zr_re")
                    zr_im = sp.tile([128, 512], BF16, tag="zr_im")
                    nc.scalar.activation(zv_re, p_zre, AF.Copy)
                    nc.scalar.activation(zv_im, p_zim, AF.Copy)
                    nc.vector.tensor_copy(out=zr_re, in_=p_rre)
                    nc.vector.tensor_copy(out=zr_im, in_=p_rim)

                    # ---- Q = Z*S4 + conj(rev Z)*D4  (all DVE, bf16 4x)
                    q_re = sp.tile([128, 512], BF16, tag="q_re")
                    q_im = sp.tile([128, 512], BF16, tag="q_im")
                    a1 = pc.tile([128, 512], BF16, tag="qa1")
                    a2 = pc.tile([128, 512], BF16, tag="qa2")
                    a3 = pc.tile([128, 512], BF16, tag="qa3")
                    nc.vector.tensor_tensor(a1, zv_re, s4re, ALU.mult)
                    nc.vector.tensor_tensor(a2, zv_im, s4im, ALU.mult)
                    nc.vector.tensor_tensor(a1, a1, a2, ALU.subtract)
                    nc.vector.tensor_tensor(a2, zr_re, d4re, ALU.mult)
                    nc.vector.tensor_tensor(a3, zr_im, d4im, ALU.mult)
                    nc.vector.tensor_tensor(a2, a2, a3, ALU.add)
                    nc.vector.tensor_tensor(q_re, a1, a2, ALU.add)
                    b1 = pc.tile([128, 512], BF16, tag="qb1")
                    b2 = pc.tile([128, 512], BF16, tag="qb2")
                    b3 = pc.tile([128, 512], BF16, tag="qb3")
                    nc.vector.tensor_tensor(b1, zv_re, s4im, ALU.mult)
                    nc.vector.tensor_tensor(b2, zv_im, s4re, ALU.mult)
                    nc.vector.tensor_tensor(b1, b1, b2, ALU.add)
                    nc.vector.tensor_tensor(b2, zr_re, d4im, ALU.mult)
                    nc.vector.tensor_tensor(b3, zr_im, d4re, ALU.mult)
                    nc.vector.tensor_tensor(b2, b2, b3, ALU.subtract)
                    nc.vector.tensor_tensor(q_im, b1, b2, ALU.add)

                    # ---- inverse (PSUM reuse: pC in p1/p2, pO in p3/p4 —
                    # all freed by this point within the same quad, keeping
                    # stage-A banks free for the next quad's pipeline)
                    pC_re = ps.tile([128, 512], F32, tag="ps_p1")
                    pC_im = ps.tile([128, 512], F32, tag="ps_p2")
                    mm_pair(pC_re, cs["gi_re"], q_re, cs["gi_imn"], q_im)
                    mm_pair(pC_im, cs["gi_im"], q_re, cs["gi_re"], q_im)
                    c0_re = pc.tile([128, 512], BF16, tag="c0_re")
                    c0_im = pc.tile([128, 512], BF16, tag="c0_im")
                    nc.scalar.activation(c0_re, pC_re, AF.Copy)
                    nc.scalar.activation(c0_im, pC_im, AF.Copy)
                    m1 = pc.tile([128, 512], BF16, tag="i_m1")
                    m2 = pc.tile([128, 512], BF16, tag="i_m2")
                    m3 = pc.tile([128, 512], BF16, tag="i_m3")
                    m4 = pc.tile([128, 512], BF16, tag="i_m4")
                    ct_re = pc.tile([128, 512], BF16, tag="ct_re")
                    ct_im = pc.tile([128, 512], BF16, tag="ct_im")
                    nc.vector.tensor_tensor(m1, c0_re, cs["ti_re"], ALU.mult)
                    nc.vector.tensor_tensor(m2, c0_im, cs["ti_im"], ALU.mult)
                    nc.vector.tensor_tensor(ct_re, m1, m2, ALU.subtract)
                    nc.vector.tensor_tensor(m3, c0_re, cs["ti_im"], ALU.mult)
                    nc.vector.tensor_tensor(m4, c0_im, cs["ti_re"], ALU.mult)
                    nc.vector.tensor_tensor(ct_im, m3, m4, ALU.add)
                    ctp_re = ps.tile([128, 512], BF16, tag="ps_ctt_re")
                    ctp_im = ps.tile([128, 512], BF16, tag="ps_ctt_im")
                    for tau in range(4):
                        ts_ = slice(128 * tau, 128 * tau + 128)
                        nc.tensor.transpose(ctp_re[:, ts_], ct_re[:, ts_],
                                            cs["ident"])
                        nc.tensor.transpose(ctp_im[:, ts_], ct_im[:, ts_],
                                            cs["ident"])
                    ctt_re = pc.tile([128, 512], BF16, tag="ctt_re")
                    ctt_im = pc.tile([128, 512], BF16, tag="ctt_im")
                    nc.vector.tensor_copy(out=ctt_re, in_=ctp_re)
                    nc.vector.tensor_copy(out=ctt_im, in_=ctp_im)
                    pO_re_t = ps.tile([128, 512], F32, tag="ps_p3")
                    pO_im_t = ps.tile([128, 512], F32, tag="ps_p4")
                    pO_re, pO_im = pO_re_t[:64, :], pO_im_t[:64, :]
                    mm_pair(pO_re, cs["fi_re"], ctt_re, cs["fi_imn"], ctt_im)
                    mm_pair(pO_im, cs["fi_im"], ctt_re, cs["fi_re"], ctt_im)
                    o_re = pc.tile([64, 512], BF16, tag="o_re")
                    o_im = pc.tile([64, 512], BF16, tag="o_im")
                    nc.scalar.activation(o_re, pO_re, AF.Copy)
                    nc.scalar.activation(o_im, pO_im, AF.Copy)
                    base = 16 * q4 * N
                    pat = [[64, 64], [4 * N, 4], [2 * N, 2], [1, 64]]
                    nc.sync.dma_start(out=ap(o_bf, base, pat), in_=o_re)
                    nc.sync.dma_start(out=ap(o_bf, base + N, pat), in_=o_im)

            # =====================================================
            # Phase D: gate + output projection (f32r partials)
            # =====================================================
            with tc.tile_pool(name="pd_w", bufs=1) as wdp, \
                 tc.tile_pool(name="pd", bufs=2) as pd, \
                 tc.tile_pool(name="pd_ps", bufs=2, space="PSUM") as dps:
                wo_sb = wdp.tile([128, 4, D], F32R, tag="wo_sb")
                nc.gpsimd.dma_start(
                    out=wo_sb, in_=ap(wo, 0, [[D, 128], [128 * D, 4], [1, D]]))
                for sb in range(8):
                    gts = []
                    for cb in range(4):
                        ut = pd.tile([128, 512], BF16, tag=f"g_u{cb}")
                        ot = pd.tile([128, 512], BF16, tag=f"g_o{cb}")
                        nc.sync.dma_start(
                            out=ut, in_=ap(u_bf, 128 * cb * N + 512 * sb,
                                           [[N, 128], [1, 512]]))
                        nc.sync.dma_start(
                            out=ot, in_=ap(o_bf, 128 * cb * N + 512 * sb,
                                           [[N, 128], [1, 512]]))
                        gt = pd.tile([128, 512], F32R, tag=f"g_g{cb}")
                        nc.vector.tensor_tensor(gt, ut, ot, ALU.mult)
                        gts.append(gt)
                    for ocblk in range(8):
                        po = dps.tile([128, 512], F32, tag="out_mm")
                        for cb in range(4):
                            nc.tensor.matmul(
                                po, wo_sb[:, cb, 128 * ocblk:128 * ocblk + 128],
                                gts[cb], start=(cb == 0), stop=(cb == 3))
                        os_ = pd.tile([128, 512], F32, tag="out_sb")
                        nc.scalar.activation(os_, po, AF.Copy)
                        nc.sync.dma_start(
                            out=ap(out, 128 * ocblk * N + 512 * sb,
                                   [[N, 128], [1, 512]]),
                            in_=os_)
    return nc


_PROGRAM_CACHE = {}


def _get_program():
    if "nc" not in _PROGRAM_CACHE:
        nc = bacc.Bacc("TRN2", target_bir_lowering=False)
        build_program(nc)
        nc.compile()
        _PROGRAM_CACHE["nc"] = nc
    return _PROGRAM_CACHE["nc"]


def kernel(x, W_uv, W_o, rpe_in_w, rpe_hid_w, rpe_ln_g, rpe_ln_b, rpe_out_w,
           decay_gamma):
    x = np.asarray(x, np.float32)
    W_uv = np.asarray(W_uv, np.float32)
    W_o = np.asarray(W_o, np.float32)

    nc = _get_program()

    shared = dict(CONSTS)
    shared["rpe_in"] = np.ascontiguousarray(rpe_in_w, np.float32)
    shared["rpe_hid"] = np.ascontiguousarray(rpe_hid_w, np.float32)
    shared["ln_g"] = np.ascontiguousarray(rpe_ln_g, np.float32)
    shared["ln_b"] = np.ascontiguousarray(rpe_ln_b, np.float32)

    in_maps = []
    for core in range(8):
        b, h = core // 2, core % 2
        c0 = h * H
        m = dict(shared)
        m["x"] = np.ascontiguousarray(x[b])
        m["wuv"] = np.ascontiguousarray(
            np.concatenate([W_uv[:, c0:c0 + H], W_uv[:, D1 + c0:D1 + c0 + H]],
                           axis=1))
        m["wo"] = np.ascontiguousarray(np.asarray(W_o, np.float32)[c0:c0 + H, :])
        m["rpeo"] = np.ascontiguousarray(np.asarray(rpe_out_w, np.float32)[:, c0:c0 + H])
        m["dg"] = np.ascontiguousarray(
            np.asarray(decay_gamma, np.float32)[None, c0:c0 + H])
        in_maps.append(m)

    import os
    kw = {}
    if os.environ.get("KERNEL_TRACE"):
        kw = dict(trace=True, tmpdir=os.environ.get("KERNEL_TRACE_DIR") or None)
    res = run_bass_kernel_spmd(nc, in_maps, core_ids=list(range(8)), **kw)
    global LAST_RESULTS
    LAST_RESULTS = res
    outs = [r["out"] for r in res.results]
    final = np.empty((B, N, D), np.float32)
    for b in range(B):
        final[b] = (outs[2 * b] + outs[2 * b + 1]).T
    return final

